# revision 9
# baseline (speedup 1.0000x reference)
"""Trainium2 Bass kernel for DilatedMDTA (dense_transformer).

Computation (per batch image X [512, 64, 64]):
  q = DW_f(fw1 @ X) ; k = DW_g(gw1 @ X) ; v = DW_h(hw1 @ X)
  where DW_* is a depthwise 3x3 dilation-2 conv with reflection pad 2.
  energy[h] = q_h @ k_h^T  (contract over the 4096 pixels)
  attn = softmax(energy * temperature, axis=-1)
  out = ow @ (attn @ v) + X

Sharding: data-parallel over batch B=16 across 8 cores (2 images/core).

Per-core mapping (v2):
  - input X is reflection-padded on the host and shipped as fp8 (e4m3);
    all four 1x1 convs run as fp8 DoubleRow matmuls (2 k-tiles/pass).
  - v-branch depthwise conv is FOLDED into the 1x1 conv on the PE for
    mts in FOLD_V_MT: 9 shifted-window accumulation passes with host
    prescaled weights W_t = diag(wd[:,t]) @ W (no elementwise tap work).
  - remaining tiles: conv psum is scattered into a padded SBUF buffer
    (ACT, scale fused), taps split DVE (tensor_scalar mul 4x + add 2x)
    and GPSIMD (scalar_tensor_tensor chain).
  - energy per head-pair as one [128]x[128] PSUM accumulation over 32
    pixel chunks of DMA-transposed qT/kT.
  - softmax: plain exp (logits are O(0.1)), 1/rowsum fused into the
    attn@v PSUM evacuation; attnout emitted as fp8 (x256) so the output
    conv also runs DoubleRow.
  - residual: identity*16384 matmul accumulates X into the ow psum
    (scales: wo x64, attnout x256 -> psum = 16384*(conv+X), evac /16384).
  - output written bf16, upcast on host.
"""

import numpy as np
import ml_dtypes

import concourse.bass as bass
from concourse import bacc
import concourse.mybir as mybir
import concourse.tile as tile
from concourse.bass import ts
from concourse.bass_utils import run_bass_kernel_spmd
from concourse.masks import make_identity

BF16 = mybir.dt.bfloat16
F32 = mybir.dt.float32
FP8 = mybir.dt.float8e4
DR = mybir.MatmulPerfMode.DoubleRow
AX = mybir.AxisListType.X
MUL = mybir.AluOpType.mult
ADD = mybir.AluOpType.add
COPY = mybir.ActivationFunctionType.Copy

N_CORES = 8
B = 16
C = 512
H = W = 64
HW = H * W
HEADS = 8
CPH = C // HEADS  # 64
P = 128
NT = C // P      # 4 channel tiles
NCH = 8          # 512-px chunks per image
NW = HW // NCH   # 512
PW = W + 4       # 68 padded width
PAD_SZ = PW * PW

SW = 64.0        # fp8 scale for unfolded 1x1 weights
SFOLD = 2048.0   # fp8 scale for folded (tap-premultiplied) weights
SATT = 256.0     # attnout scale (fused into v evacuation)
SRES = SW * SATT  # net scale of the ow psum (16384)

FOLD_V_MT = (0, 1, 2)   # v-branch mts whose DW is folded into the PE conv
GPS_TAPS = (6, 7, 8)    # taps done on GPSIMD for elementwise tiles
ACT_TAP0_MT = (0, 2)    # mts whose tap-0 product is offloaded to ACT


def _r(ap, spec, **kw):
    return ap.rearrange(spec, **kw)


def build_module(b_loc: int):
    nc = bacc.Bacc("TRN2", target_bir_lowering=False, debug=False)

    xp8 = nc.dram_tensor("xp8", [b_loc, P, NT * PAD_SZ], FP8, kind="ExternalInput").ap()
    xres = nc.dram_tensor("xres", [b_loc, C, HW], BF16, kind="ExternalInput").ap()
    wq = nc.dram_tensor("wq", [P, NT * C], FP8, kind="ExternalInput").ap()
    wk = nc.dram_tensor("wk", [P, NT * C], FP8, kind="ExternalInput").ap()
    wv = nc.dram_tensor("wv", [P, NT * C], FP8, kind="ExternalInput").ap()
    wfold = nc.dram_tensor("wfold", [P, 9 * NT * C], FP8, kind="ExternalInput").ap()
    wo = nc.dram_tensor("wo", [P, NT * C], FP8, kind="ExternalInput").ap()
    wd = nc.dram_tensor("wd", [P, 3 * NT * 9], F32, kind="ExternalInput").ap()
    sq = nc.dram_tensor("sq", [P, NT], F32, kind="ExternalInput").ap()
    out = nc.dram_tensor("out", [b_loc, C, HW], BF16, kind="ExternalOutput").ap()

    with tile.TileContext(nc) as tc:
        _body(tc, b_loc, xp8, xres, [wq, wk, wv], wfold, wo, wd, sq, out)
    nc.compile()
    return nc


def _body(tc, b_loc, xp8, xres, wqkv, wfold, wo, wd, sq, out):
    nc = tc.nc

    pools = []

    def mkpool(**kw):
        p = tc.alloc_tile_pool(**kw)
        pools.append(p)
        return p

    const = mkpool(name="const", bufs=1)
    x8_pool = mkpool(name="x8", bufs=2)
    xr_pool = mkpool(name="xr", bufs=1)
    xpad_pool = mkpool(name="xpad", bufs=2)
    qk_pool = mkpool(name="qk", bufs=2)
    v_pool = mkpool(name="v", bufs=2)
    qt_pool = mkpool(name="qt", bufs=1)
    att_pool = mkpool(name="att", bufs=1)
    small_pool = mkpool(name="small", bufs=2)
    prod_dve = mkpool(name="prodd", bufs=2)
    prod_act = mkpool(name="proda", bufs=1)
    outp = mkpool(name="outp", bufs=2)
    ps_fold = mkpool(name="ps_fold", bufs=2, space="PSUM")
    ps_conv = mkpool(name="ps_conv", bufs=2, space="PSUM")
    ps_e = mkpool(name="ps_e", bufs=1, space="PSUM")
    ps_t = mkpool(name="ps_t", bufs=1, space="PSUM")

    # weights / consts
    w_sb = []
    for name, wdram in zip("qkv", wqkv):
        t = const.tile([P, NT, C], FP8, tag=f"w{name}")
        nc.sync.dma_start(t[:], _r(wdram, "p (kt o) -> p kt o", kt=NT))
        w_sb.append(t)
    wfold_sb = const.tile([P, 9, NT, C], FP8, tag="wfold")
    nc.sync.dma_start(wfold_sb[:], _r(wfold, "p (t kt o) -> p t kt o", t=9, kt=NT))
    wo_sb = const.tile([P, NT, C], FP8, tag="wo")
    nc.sync.dma_start(wo_sb[:], _r(wo, "p (kt o) -> p kt o", kt=NT))
    wd_sb = const.tile([P, 3 * NT * 9], F32, tag="wd")
    nc.sync.dma_start(wd_sb[:], wd[:])
    sq_sb = const.tile([P, NT], F32, tag="sq")
    nc.sync.dma_start(sq_sb[:], sq[:])
    ident = const.tile([P, P], BF16, tag="ident")
    make_identity(nc, ident[:])
    identr = const.tile([P, P], BF16, tag="identr")
    nc.scalar.mul(identr[:], ident[:], float(SRES))

    def conv_scatter(br, mt, xp8v, xpv):
        """unfolded 1x1 conv (fp8 DR) -> evac-scatter into padded buffer."""
        scale = sq_sb[:, mt : mt + 1] if br == 0 else (SATT / SW if br == 2 else 1.0 / SW)
        for ch in range(NCH):
            ps = ps_conv.tile([P, NW], F32, tag="cps")
            psv = _r(ps[:], "p (r c) -> p r c", c=W)
            r0 = 8 * ch
            for i in range(2):
                nc.tensor.matmul(
                    psv,
                    w_sb[br][:, 2 * i : 2 * i + 2, ts(mt, P)],
                    xp8v[:, 2 * i : 2 * i + 2, r0 + 2 : r0 + 10, 2 : 2 + W],
                    start=(i == 0),
                    stop=(i == 1),
                    perf_mode=DR,
                )
            dst = xpv[:, 2 + r0 : 10 + r0, 2 : 2 + W]
            nc.scalar.activation(dst, psv, COPY, scale=scale)
            if ch == 0:
                nc.scalar.activation(xpv[:, 0:1, 2 : 2 + W], psv[:, 2:3], COPY, scale=scale)
                nc.scalar.activation(xpv[:, 1:2, 2 : 2 + W], psv[:, 1:2], COPY, scale=scale)
            if ch == NCH - 1:
                nc.scalar.activation(xpv[:, 66:67, 2 : 2 + W], psv[:, 6:7], COPY, scale=scale)
                nc.scalar.activation(xpv[:, 67:68, 2 : 2 + W], psv[:, 5:6], COPY, scale=scale)
        # full-height column reflection pads on DVE
        nc.vector.tensor_copy(xpv[:, :, 0:1], xpv[:, :, 4:5])
        nc.vector.tensor_copy(xpv[:, :, 1:2], xpv[:, :, 3:4])
        nc.vector.tensor_copy(xpv[:, :, 66:67], xpv[:, :, 64:65])
        nc.vector.tensor_copy(xpv[:, :, 67:68], xpv[:, :, 63:64])

    def dw_taps(br, mt, xpv, y, n_gps):
        """9-tap depthwise: muls on DVE (4x) + one on ACT; adds split
        DVE (2x) / GPSIMD (tensor_tensor only — STT is not a Pool op)."""

        def wsc(t):
            i = (br * NT + mt) * 9 + t
            return wd_sb[:, i : i + 1]

        def srcf(t):
            i, j = t // 3, t % 3
            return xpv[:, 2 * i : 2 * i + H, 2 * j : 2 * j + W]

        yv = _r(y[:], "p (r c) -> p r c", c=W)
        # tap 0 initializes y on DVE; tap 1 product on ACT; products for
        # the GPSIMD-added taps come last so their adds tail the chain.
        nc.vector.tensor_scalar_mul(yv, srcf(0), wsc(0))
        pf0 = prod_act.tile([P, HW], BF16, tag="pf0")
        nc.scalar.activation(
            _r(pf0[:], "p (r c) -> p r c", c=W), srcf(1), COPY, scale=wsc(1)
        )
        for t in range(2, 9):
            pf = prod_dve.tile([P, HW], BF16, tag="pf")
            nc.vector.tensor_scalar_mul(_r(pf[:], "p (r c) -> p r c", c=W), srcf(t), wsc(t))
            eng = nc.gpsimd if t >= 9 - n_gps else nc.vector
            eng.tensor_add(y[:], y[:], pf[:])
        nc.vector.tensor_add(y[:], y[:], pf0[:])

    def folded_tile(mt, xp8v, y):
        """v-branch conv+DW fused on the PE: 9 shifted-window DR passes."""
        for ci in range(4):
            ps = ps_fold.tile([P, 2, NW], F32, tag="fps")
            for t in range(9):
                ir, jc = t // 3, t % 3
                for i in range(2):
                    lw = wfold_sb[:, t, 2 * i : 2 * i + 2, ts(mt, P)]
                    for c2 in range(2):
                        r0 = 8 * (ci * 2 + c2)
                        nc.tensor.matmul(
                            _r(ps[:, c2], "p (r c) -> p r c", c=W),
                            lw,
                            xp8v[:, 2 * i : 2 * i + 2, r0 + 2 * ir : r0 + 2 * ir + 8, 2 * jc : 2 * jc + W],
                            start=(t == 0 and i == 0),
                            stop=(t == 8 and i == 1),
                            perf_mode=DR,
                        )
            nc.scalar.activation(
                y[:, ci * 2 * NW : (ci + 1) * 2 * NW],
                _r(ps[:], "p a b -> p (a b)"),
                COPY,
                scale=SATT / SFOLD,
            )

    def attention(mt, qT, kT, v, attnout):
        # energy for head pair (2*mt, 2*mt+1); head-cross blocks unused
        eps = ps_e.tile([P, P], F32, tag="eps")
        for nk in range(32):
            nc.tensor.matmul(
                eps[:], qT[:, nk], kT[:, nk], start=(nk == 0), stop=(nk == 31)
            )
        s = small_pool.tile([P, 1], F32, tag="s")
        r = small_pool.tile([P, 1], F32, tag="r")
        exps = small_pool.tile([P, P], BF16, tag="exps")
        # energies here are O(0.1): plain exp is safe, no max subtraction
        nc.scalar.activation(
            exps[:], eps[:], mybir.ActivationFunctionType.Exp, bias=0.0, scale=1.0
        )
        for hh in range(2):
            h0 = CPH * hh
            nc.vector.reduce_sum(
                s[h0 : h0 + CPH], exps[h0 : h0 + CPH, h0 : h0 + CPH], axis=AX
            )
            nc.vector.reciprocal(r[h0 : h0 + CPH], s[h0 : h0 + CPH])

        tps = ps_t.tile([P, P], BF16, tag="tps")
        nc.tensor.transpose(tps[:], exps[:], ident[:])
        attnT = small_pool.tile([P, P], BF16, tag="attnT")
        nc.scalar.copy(attnT[:], tps[:])

        # attn @ v: both heads into one psum bank (concurrent quadrants),
        # single evacuation scaled by 1/rowsum (v carries x256 already)
        for nch in range(NCH):
            pa = ps_conv.tile([P, NW], F32, tag="cps")
            for hh in range(2):
                h0 = CPH * hh
                nc.tensor.matmul(
                    pa[h0 : h0 + CPH],
                    attnT[h0 : h0 + CPH, h0 : h0 + CPH],
                    v[h0 : h0 + CPH, ts(nch, NW)],
                    start=True,
                    stop=True,
                    tile_position=(h0, h0),
                )
            nc.scalar.activation(
                attnout[:, mt, ts(nch, NW)], pa[:], COPY, scale=r[:]
            )

    def ow_block(b, attnout, xrt):
        for mt in range(NT):
            for nch in range(NCH):
                ps = ps_conv.tile([P, NW], F32, tag="cps")
                for i in range(2):
                    nc.tensor.matmul(
                        ps[:],
                        wo_sb[:, 2 * i : 2 * i + 2, ts(mt, P)],
                        attnout[:, 2 * i : 2 * i + 2, ts(nch, NW)],
                        start=(i == 0),
                        stop=False,
                        perf_mode=DR,
                    )
                # residual: += 16384 * X  (identity premultiplied by SRES)
                nc.tensor.matmul(
                    ps[:], identr[:], xrt[:, mt, ts(nch, NW)],
                    start=False, stop=True,
                )
                ot = outp.tile([P, NW], BF16, tag="ot")
                nc.scalar.activation(ot[:], ps[:], COPY, scale=1.0 / SRES)
                nc.sync.dma_start(out[b, ts(mt, P), ts(nch, NW)], ot[:])

    pending_ow = None
    for b in range(b_loc):
        x8t = x8_pool.tile([P, NT, PAD_SZ], FP8, tag="x8t")
        nc.sync.dma_start(_r(x8t[:], "p kt s -> p (kt s)"), xp8[b])
        xp8v = _r(x8t[:], "p kt (r c) -> p kt r c", c=PW)
        xrt = xr_pool.tile([P, NT, HW], BF16, tag="xrt")
        nc.sync.dma_start(xrt[:], _r(xres[b], "(kt p) n -> p kt n", p=P))

        attnout = att_pool.tile([P, NT, HW], FP8, tag="attnout")
        pending = None  # deferred attention block for software pipelining

        for mt in range(NT):
            if mt == 1 and pending_ow is not None:
                ow_block(*pending_ow)
                pending_ow = None
            ydw = {}
            for br in range(3):
                pool = v_pool if br == 2 else qk_pool
                y = pool.tile([P, HW], BF16, tag="v" if br == 2 else "qk")
                if br == 2 and mt in FOLD_V_MT:
                    folded_tile(mt, xp8v, y)
                else:
                    xpad = xpad_pool.tile([P, PAD_SZ], BF16, tag="xpad")
                    xpv = _r(xpad[:], "p (r c) -> p r c", c=PW)
                    # for v the SATT scale rides the conv evacuation (taps
                    # are linear), so y is 256*v in every path
                    conv_scatter(br, mt, xp8v, xpv)
                    dw_taps(br, mt, xpv, y, n_gps=2)
                ydw[br] = y

            qT = qt_pool.tile([P, 32, P], BF16, tag="qT")
            kT = qt_pool.tile([P, 32, P], BF16, tag="kT")
            for qq in range(8):
                nc.sync.dma_start_transpose(
                    qT[:, qq * 4 : (qq + 1) * 4], ydw[0][:, ts(qq, 512)]
                )
                nc.sync.dma_start_transpose(
                    kT[:, qq * 4 : (qq + 1) * 4], ydw[1][:, ts(qq, 512)]
                )

            if pending is not None:
                attention(*pending)
            pending = (mt, qT, kT, ydw[2], attnout)

        attention(*pending)
        pending_ow = (b, attnout, xrt)

    ow_block(*pending_ow)

    for p in reversed(pools):
        p.release()


def _fp8(a):
    return np.clip(np.asarray(a, np.float32), -240.0, 240.0).astype(
        ml_dtypes.float8_e4m3
    )


def prep_inputs(style_feat, fw1, fwd_, gw1, gwd, hw1, hwd, ow, temperature):
    """Host-side prep: pad+quantize input, prescale weights, shard over batch."""
    bf16 = ml_dtypes.bfloat16
    sf = np.asarray(style_feat, np.float32)
    temp = np.asarray(temperature, np.float32).reshape(HEADS)

    # padded fp8 input: [B, P, NT*PAD_SZ]
    xpad = np.pad(sf, ((0, 0), (0, 0), (2, 2), (2, 2)), mode="reflect")
    xpad = xpad.reshape(B, NT, P, PAD_SZ).transpose(0, 2, 1, 3).reshape(B, P, NT * PAD_SZ)
    xp8 = _fp8(xpad)

    xres = sf.reshape(B, C, HW).astype(bf16)

    def wT(m, scale):  # [P, NT*C]: [p, kt, o] = m[o, kt*128+p] * scale
        a = (np.asarray(m, np.float32).T * scale).reshape(NT, P, C)
        return a.transpose(1, 0, 2).reshape(P, NT * C)

    wq8 = _fp8(wT(fw1, SW))
    wk8 = _fp8(wT(gw1, SW))
    wv8 = _fp8(wT(hw1, SW))
    wo8 = _fp8(wT(ow, SW))

    # folded v weights: [p, t, kt, o] = hwd[o,t] * hw1[o, kt*128+p] * SFOLD
    wd_v = np.asarray(hwd, np.float32).reshape(C, 9)
    m = np.asarray(hw1, np.float32)
    a = np.einsum("ot,ok->tko", wd_v, m) * SFOLD  # [9, C_in, C_out]
    a = a.reshape(9, NT, P, C).transpose(2, 0, 1, 3).reshape(P, 9 * NT * C)
    wfold8 = _fp8(a)

    # depthwise tap weights -> [128, branch*ctile*9]
    wd_all = np.zeros((P, 3 * NT * 9), dtype=np.float32)
    for bi, wdb in enumerate([fwd_, gwd, hwd]):
        wdb = np.asarray(wdb, np.float32).reshape(C, 9)
        for mt in range(NT):
            wd_all[:, (bi * NT + mt) * 9 : (bi * NT + mt) * 9 + 9] = wdb[
                mt * P : (mt + 1) * P
            ]

    # q evacuation scale: temp per output channel / SW
    tvec = np.repeat(temp, CPH)  # [C]
    sq = (tvec / SW).reshape(NT, P).T.copy()  # [P, NT]

    b_loc = B // N_CORES
    in_maps = []
    for ci in range(N_CORES):
        sl = slice(ci * b_loc, (ci + 1) * b_loc)
        in_maps.append(
            dict(
                xp8=np.ascontiguousarray(xp8[sl]),
                xres=np.ascontiguousarray(xres[sl]),
                wq=wq8, wk=wk8, wv=wv8, wfold=wfold8, wo=wo8,
                wd=wd_all, sq=np.ascontiguousarray(sq),
            )
        )
    return in_maps, b_loc


_CACHED = {}


def _get_module(b_loc):
    if b_loc not in _CACHED:
        _CACHED[b_loc] = build_module(b_loc)
    return _CACHED[b_loc]


def kernel(**inputs):
    in_maps, b_loc = prep_inputs(**inputs)
    nc = _get_module(b_loc)
    res = run_bass_kernel_spmd(nc, in_maps, list(range(N_CORES)))
    outs = [np.asarray(res.results[i]["out"]) for i in range(N_CORES)]
    full = np.concatenate(outs, axis=0).reshape(B, C, H, W)
    return full.astype(np.float32)


if __name__ == "__main__":
    rng = np.random.default_rng(0)
    inputs = dict(
        style_feat=rng.standard_normal((B, C, H, W), dtype=np.float32),
        fw1=(rng.standard_normal((C, C), dtype=np.float32) * 0.02),
        fwd_=(rng.standard_normal((C, 1, 3, 3), dtype=np.float32) * 0.02),
        gw1=(rng.standard_normal((C, C), dtype=np.float32) * 0.02),
        gwd=(rng.standard_normal((C, 1, 3, 3), dtype=np.float32) * 0.02),
        hw1=(rng.standard_normal((C, C), dtype=np.float32) * 0.02),
        hwd=(rng.standard_normal((C, 1, 3, 3), dtype=np.float32) * 0.02),
        ow=(rng.standard_normal((C, C), dtype=np.float32) * 0.02),
        temperature=np.ones((HEADS, 1, 1), dtype=np.float32),
    )
    o = kernel(**inputs)
    print(o.shape, o.dtype)


# revision 11
# speedup vs baseline: 1.4235x; 1.4235x over previous
"""Trainium2 Bass kernel for DilatedMDTA (dense_transformer).

Computation (per batch image X [512, 64, 64]):
  q = DW_f(fw1 @ X) ; k = DW_g(gw1 @ X) ; v = DW_h(hw1 @ X)
  where DW_* is a depthwise 3x3 dilation-2 conv with reflection pad 2.
  energy[h] = q_h @ k_h^T  (contract over the 4096 pixels)
  attn = softmax(energy * temperature, axis=-1)
  out = ow @ (attn @ v) + X

Sharding: data-parallel over batch B=16 across 8 cores (2 images/core).

Per-core mapping (v2):
  - input X is reflection-padded on the host and shipped as fp8 (e4m3);
    all four 1x1 convs run as fp8 DoubleRow matmuls (2 k-tiles/pass).
  - v-branch depthwise conv is FOLDED into the 1x1 conv on the PE for
    mts in FOLD_V_MT: 9 shifted-window accumulation passes with host
    prescaled weights W_t = diag(wd[:,t]) @ W (no elementwise tap work).
  - remaining tiles: conv psum is scattered into a padded SBUF buffer
    (ACT, scale fused), taps split DVE (tensor_scalar mul 4x + add 2x)
    and GPSIMD (scalar_tensor_tensor chain).
  - energy per head-pair as one [128]x[128] PSUM accumulation over 32
    pixel chunks of DMA-transposed qT/kT.
  - softmax: plain exp (logits are O(0.1)), 1/rowsum fused into the
    attn@v PSUM evacuation; attnout emitted as fp8 (x256) so the output
    conv also runs DoubleRow.
  - residual: identity*16384 matmul accumulates X into the ow psum
    (scales: wo x64, attnout x256 -> psum = 16384*(conv+X), evac /16384).
  - output written bf16, upcast on host.
"""

import numpy as np
import ml_dtypes

import concourse.bass as bass
from concourse import bacc
import concourse.mybir as mybir
import concourse.tile as tile
from concourse.bass import ts
from concourse.bass_utils import run_bass_kernel_spmd
from concourse.masks import make_identity

BF16 = mybir.dt.bfloat16
F32 = mybir.dt.float32
FP8 = mybir.dt.float8e4
DR = mybir.MatmulPerfMode.DoubleRow
AX = mybir.AxisListType.X
MUL = mybir.AluOpType.mult
ADD = mybir.AluOpType.add
COPY = mybir.ActivationFunctionType.Copy

N_CORES = 8
B = 16
C = 512
H = W = 64
HW = H * W
HEADS = 8
CPH = C // HEADS  # 64
P = 128
NT = C // P      # 4 channel tiles
NCH = 8          # 512-px chunks per image
NW = HW // NCH   # 512
PW = W + 4       # 68 padded width
PAD_SZ = PW * PW

SW = 64.0        # fp8 scale for unfolded 1x1 weights
SFOLD = 2048.0   # fp8 scale for folded (tap-premultiplied) weights
SATT = 256.0     # attnout scale (fused into v evacuation)
SRES = SW * SATT  # net scale of the ow psum (16384)

FOLD_V_MT = (0, 1, 2)   # v-branch mts whose DW is folded into the PE conv
GPS_TAPS = (6, 7, 8)    # taps done on GPSIMD for elementwise tiles
ACT_TAP0_MT = (0, 2)    # mts whose tap-0 product is offloaded to ACT


def _r(ap, spec, **kw):
    return ap.rearrange(spec, **kw)


def build_module(b_loc: int):
    nc = bacc.Bacc("TRN2", target_bir_lowering=False, debug=False)

    xp8 = nc.dram_tensor("xp8", [b_loc, P, NT * PAD_SZ], FP8, kind="ExternalInput").ap()
    xres = nc.dram_tensor("xres", [b_loc, C, HW], BF16, kind="ExternalInput").ap()
    wq = nc.dram_tensor("wq", [P, NT * C], FP8, kind="ExternalInput").ap()
    wk = nc.dram_tensor("wk", [P, NT * C], FP8, kind="ExternalInput").ap()
    wv = nc.dram_tensor("wv", [P, NT * C], FP8, kind="ExternalInput").ap()
    wfold = nc.dram_tensor("wfold", [P, 9 * NT * C], FP8, kind="ExternalInput").ap()
    wo = nc.dram_tensor("wo", [P, NT * C], FP8, kind="ExternalInput").ap()
    wd = nc.dram_tensor("wd", [P, 3 * NT * 9], F32, kind="ExternalInput").ap()
    sq = nc.dram_tensor("sq", [P, NT], F32, kind="ExternalInput").ap()
    out = nc.dram_tensor("out", [b_loc, C, HW], BF16, kind="ExternalOutput").ap()

    with tile.TileContext(nc) as tc:
        _body(tc, b_loc, xp8, xres, [wq, wk, wv], wfold, wo, wd, sq, out)
    nc.compile()
    return nc


def _body(tc, b_loc, xp8, xres, wqkv, wfold, wo, wd, sq, out):
    nc = tc.nc

    pools = []

    def mkpool(**kw):
        p = tc.alloc_tile_pool(**kw)
        pools.append(p)
        return p

    const = mkpool(name="const", bufs=1)
    x8_pool = mkpool(name="x8", bufs=2)
    xr_pool = mkpool(name="xr", bufs=1)
    xpad_pool = mkpool(name="xpad", bufs=2)
    qk_pool = mkpool(name="qk", bufs=2)
    v_pool = mkpool(name="v", bufs=2)
    qt_pool = mkpool(name="qt", bufs=1)
    att_pool = mkpool(name="att", bufs=1)
    small_pool = mkpool(name="small", bufs=2)
    prod_dve = mkpool(name="prodd", bufs=2)
    prod_act = mkpool(name="proda", bufs=1)
    outp = mkpool(name="outp", bufs=2)
    ps_fold = mkpool(name="ps_fold", bufs=2, space="PSUM")
    ps_conv = mkpool(name="ps_conv", bufs=2, space="PSUM")
    ps_e = mkpool(name="ps_e", bufs=1, space="PSUM")
    ps_t = mkpool(name="ps_t", bufs=1, space="PSUM")

    # weights / consts
    w_sb = []
    for name, wdram in zip("qkv", wqkv):
        t = const.tile([P, NT, C], FP8, tag=f"w{name}")
        nc.sync.dma_start(t[:], _r(wdram, "p (kt o) -> p kt o", kt=NT))
        w_sb.append(t)
    wfold_sb = const.tile([P, 9, NT, C], FP8, tag="wfold")
    nc.sync.dma_start(wfold_sb[:], _r(wfold, "p (t kt o) -> p t kt o", t=9, kt=NT))
    wo_sb = const.tile([P, NT, C], FP8, tag="wo")
    nc.sync.dma_start(wo_sb[:], _r(wo, "p (kt o) -> p kt o", kt=NT))
    wd_sb = const.tile([P, 3 * NT * 9], F32, tag="wd")
    nc.sync.dma_start(wd_sb[:], wd[:])
    sq_sb = const.tile([P, NT], F32, tag="sq")
    nc.sync.dma_start(sq_sb[:], sq[:])
    ident = const.tile([P, P], BF16, tag="ident")
    make_identity(nc, ident[:])
    identr = const.tile([P, P], BF16, tag="identr")
    nc.scalar.mul(identr[:], ident[:], float(SRES))

    def conv_scatter(br, mt, x8t, xpad):
        """1x1 conv of the full PADDED input (pointwise: conv(pad(X)) =
        pad(conv(X))) -> padded buffer with no pad copies at all.
        10 chunks of 7 padded rows (last 5), contiguous in and out."""
        scale = sq_sb[:, mt : mt + 1] if br == 0 else (SATT / SW if br == 2 else 1.0 / SW)
        for ch in range(10):
            r0 = 7 * ch
            ncol = (5 if ch == 9 else 7) * PW
            ps = ps_conv.tile([P, NW], F32, tag="cps")
            for i in range(2):
                nc.tensor.matmul(
                    ps[:, 0:ncol],
                    w_sb[br][:, 2 * i : 2 * i + 2, ts(mt, P)],
                    x8t[:, 2 * i : 2 * i + 2, r0 * PW : r0 * PW + ncol],
                    start=(i == 0),
                    stop=(i == 1),
                    perf_mode=DR,
                )
            nc.scalar.activation(
                xpad[:, r0 * PW : r0 * PW + ncol], ps[:, 0:ncol], COPY, scale=scale
            )

    def dw_taps(br, mt, xpv, y):
        """9-tap depthwise: 2 muls on ACT (incl. the y init), the rest
        muls at 4x + all adds at 2x on DVE. GPSIMD stays idle: its ops
        steal the SBUF port DVE needs for 2x/4x modes."""

        def wsc(t):
            i = (br * NT + mt) * 9 + t
            return wd_sb[:, i : i + 1]

        def srcf(t):
            i, j = t // 3, t % 3
            return xpv[:, 2 * i : 2 * i + H, 2 * j : 2 * j + W]

        yv = _r(y[:], "p (r c) -> p r c", c=W)
        # ACT initializes y with tap 0 and makes the tap-1 product
        nc.scalar.activation(yv, srcf(0), COPY, scale=wsc(0))
        pf0 = prod_act.tile([P, HW], BF16, tag="pf0")
        nc.scalar.activation(
            _r(pf0[:], "p (r c) -> p r c", c=W), srcf(1), COPY, scale=wsc(1)
        )
        for t in range(2, 9):
            pf = prod_dve.tile([P, HW], BF16, tag="pf")
            nc.vector.tensor_scalar_mul(_r(pf[:], "p (r c) -> p r c", c=W), srcf(t), wsc(t))
            nc.vector.tensor_add(y[:], y[:], pf[:])
        nc.vector.tensor_add(y[:], y[:], pf0[:])

    def folded_tile(mt, xp8v, y):
        """v-branch conv+DW fused on the PE: 9 shifted-window DR passes."""
        for ci in range(4):
            ps = ps_fold.tile([P, 2, NW], F32, tag="fps")
            for t in range(9):
                ir, jc = t // 3, t % 3
                for i in range(2):
                    lw = wfold_sb[:, t, 2 * i : 2 * i + 2, ts(mt, P)]
                    for c2 in range(2):
                        r0 = 8 * (ci * 2 + c2)
                        nc.tensor.matmul(
                            _r(ps[:, c2], "p (r c) -> p r c", c=W),
                            lw,
                            xp8v[:, 2 * i : 2 * i + 2, r0 + 2 * ir : r0 + 2 * ir + 8, 2 * jc : 2 * jc + W],
                            start=(t == 0 and i == 0),
                            stop=(t == 8 and i == 1),
                            perf_mode=DR,
                        )
            nc.scalar.activation(
                y[:, ci * 2 * NW : (ci + 1) * 2 * NW],
                _r(ps[:], "p a b -> p (a b)"),
                COPY,
                scale=SATT / SFOLD,
            )

    def attention(mt, qT, kT, v, attnout):
        # energy for head pair (2*mt, 2*mt+1); head-cross blocks unused
        eps = ps_e.tile([P, P], F32, tag="eps")
        for nk in range(32):
            nc.tensor.matmul(
                eps[:], qT[:, nk], kT[:, nk], start=(nk == 0), stop=(nk == 31)
            )
        s = small_pool.tile([P, 1], F32, tag="s")
        r = small_pool.tile([P, 1], F32, tag="r")
        exps = small_pool.tile([P, P], BF16, tag="exps")
        # energies here are O(0.1): plain exp is safe, no max subtraction
        nc.scalar.activation(
            exps[:], eps[:], mybir.ActivationFunctionType.Exp, bias=0.0, scale=1.0
        )
        for hh in range(2):
            h0 = CPH * hh
            nc.vector.reduce_sum(
                s[h0 : h0 + CPH], exps[h0 : h0 + CPH, h0 : h0 + CPH], axis=AX
            )
            nc.vector.reciprocal(r[h0 : h0 + CPH], s[h0 : h0 + CPH])

        tps = ps_t.tile([P, P], BF16, tag="tps")
        nc.tensor.transpose(tps[:], exps[:], ident[:])
        attnT = small_pool.tile([P, P], BF16, tag="attnT")
        nc.scalar.copy(attnT[:], tps[:])

        # attn @ v: both heads into one psum bank (concurrent quadrants),
        # single evacuation scaled by 1/rowsum (v carries x256 already)
        for nch in range(NCH):
            pa = ps_conv.tile([P, NW], F32, tag="cps")
            for hh in range(2):
                h0 = CPH * hh
                nc.tensor.matmul(
                    pa[h0 : h0 + CPH],
                    attnT[h0 : h0 + CPH, h0 : h0 + CPH],
                    v[h0 : h0 + CPH, ts(nch, NW)],
                    start=True,
                    stop=True,
                    tile_position=(h0, h0),
                )
            nc.scalar.activation(
                attnout[:, mt, ts(nch, NW)], pa[:], COPY, scale=r[:]
            )

    def ow_block(b, attnout, xrt):
        for mt in range(NT):
            for nch in range(NCH):
                ps = ps_conv.tile([P, NW], F32, tag="cps")
                for i in range(2):
                    nc.tensor.matmul(
                        ps[:],
                        wo_sb[:, 2 * i : 2 * i + 2, ts(mt, P)],
                        attnout[:, 2 * i : 2 * i + 2, ts(nch, NW)],
                        start=(i == 0),
                        stop=False,
                        perf_mode=DR,
                    )
                # residual: += 16384 * X  (identity premultiplied by SRES)
                nc.tensor.matmul(
                    ps[:], identr[:], xrt[:, mt, ts(nch, NW)],
                    start=False, stop=True,
                )
                ot = outp.tile([P, NW], BF16, tag="ot")
                nc.scalar.activation(ot[:], ps[:], COPY, scale=1.0 / SRES)
                nc.sync.dma_start(out[b, ts(mt, P), ts(nch, NW)], ot[:])

    pending_ow = None
    for b in range(b_loc):
        x8t = x8_pool.tile([P, NT, PAD_SZ], FP8, tag="x8t")
        nc.sync.dma_start(_r(x8t[:], "p kt s -> p (kt s)"), xp8[b])
        xp8v = _r(x8t[:], "p kt (r c) -> p kt r c", c=PW)
        xrt = xr_pool.tile([P, NT, HW], BF16, tag="xrt")
        nc.sync.dma_start(xrt[:], _r(xres[b], "(kt p) n -> p kt n", p=P))

        attnout = att_pool.tile([P, NT, HW], FP8, tag="attnout")
        pending = None  # deferred attention block for software pipelining

        for mt in range(NT):
            if mt == 1 and pending_ow is not None:
                ow_block(*pending_ow)
                pending_ow = None
            ydw = {}
            for br in range(3):
                pool = v_pool if br == 2 else qk_pool
                y = pool.tile([P, HW], BF16, tag="v" if br == 2 else "qk")
                if br == 2 and mt in FOLD_V_MT:
                    folded_tile(mt, xp8v, y)
                else:
                    xpad = xpad_pool.tile([P, PAD_SZ], BF16, tag="xpad")
                    xpv = _r(xpad[:], "p (r c) -> p r c", c=PW)
                    # for v the SATT scale rides the conv evacuation (taps
                    # are linear), so y is 256*v in every path
                    conv_scatter(br, mt, x8t, xpad[:])
                    dw_taps(br, mt, xpv, y)
                ydw[br] = y

            qT = qt_pool.tile([P, 32, P], BF16, tag="qT")
            kT = qt_pool.tile([P, 32, P], BF16, tag="kT")
            for qq in range(8):
                nc.sync.dma_start_transpose(
                    qT[:, qq * 4 : (qq + 1) * 4], ydw[0][:, ts(qq, 512)]
                )
                nc.sync.dma_start_transpose(
                    kT[:, qq * 4 : (qq + 1) * 4], ydw[1][:, ts(qq, 512)]
                )

            if pending is not None:
                attention(*pending)
            pending = (mt, qT, kT, ydw[2], attnout)

        attention(*pending)
        pending_ow = (b, attnout, xrt)

    ow_block(*pending_ow)

    for p in reversed(pools):
        p.release()


def _fp8(a):
    return np.clip(np.asarray(a, np.float32), -240.0, 240.0).astype(
        ml_dtypes.float8_e4m3
    )


def prep_inputs(style_feat, fw1, fwd_, gw1, gwd, hw1, hwd, ow, temperature):
    """Host-side prep: pad+quantize input, prescale weights, shard over batch."""
    bf16 = ml_dtypes.bfloat16
    sf = np.asarray(style_feat, np.float32)
    temp = np.asarray(temperature, np.float32).reshape(HEADS)

    # padded fp8 input: [B, P, NT*PAD_SZ]
    xpad = np.pad(sf, ((0, 0), (0, 0), (2, 2), (2, 2)), mode="reflect")
    xpad = xpad.reshape(B, NT, P, PAD_SZ).transpose(0, 2, 1, 3).reshape(B, P, NT * PAD_SZ)
    xp8 = _fp8(xpad)

    xres = sf.reshape(B, C, HW).astype(bf16)

    def wT(m, scale):  # [P, NT*C]: [p, kt, o] = m[o, kt*128+p] * scale
        a = (np.asarray(m, np.float32).T * scale).reshape(NT, P, C)
        return a.transpose(1, 0, 2).reshape(P, NT * C)

    wq8 = _fp8(wT(fw1, SW))
    wk8 = _fp8(wT(gw1, SW))
    wv8 = _fp8(wT(hw1, SW))
    wo8 = _fp8(wT(ow, SW))

    # folded v weights: [p, t, kt, o] = hwd[o,t] * hw1[o, kt*128+p] * SFOLD
    wd_v = np.asarray(hwd, np.float32).reshape(C, 9)
    m = np.asarray(hw1, np.float32)
    a = np.einsum("ot,ok->tko", wd_v, m) * SFOLD  # [9, C_in, C_out]
    a = a.reshape(9, NT, P, C).transpose(2, 0, 1, 3).reshape(P, 9 * NT * C)
    wfold8 = _fp8(a)

    # depthwise tap weights -> [128, branch*ctile*9]
    wd_all = np.zeros((P, 3 * NT * 9), dtype=np.float32)
    for bi, wdb in enumerate([fwd_, gwd, hwd]):
        wdb = np.asarray(wdb, np.float32).reshape(C, 9)
        for mt in range(NT):
            wd_all[:, (bi * NT + mt) * 9 : (bi * NT + mt) * 9 + 9] = wdb[
                mt * P : (mt + 1) * P
            ]

    # q evacuation scale: temp per output channel / SW
    tvec = np.repeat(temp, CPH)  # [C]
    sq = (tvec / SW).reshape(NT, P).T.copy()  # [P, NT]

    b_loc = B // N_CORES
    in_maps = []
    for ci in range(N_CORES):
        sl = slice(ci * b_loc, (ci + 1) * b_loc)
        in_maps.append(
            dict(
                xp8=np.ascontiguousarray(xp8[sl]),
                xres=np.ascontiguousarray(xres[sl]),
                wq=wq8, wk=wk8, wv=wv8, wfold=wfold8, wo=wo8,
                wd=wd_all, sq=np.ascontiguousarray(sq),
            )
        )
    return in_maps, b_loc


_CACHED = {}


def _get_module(b_loc):
    if b_loc not in _CACHED:
        _CACHED[b_loc] = build_module(b_loc)
    return _CACHED[b_loc]


def kernel(**inputs):
    in_maps, b_loc = prep_inputs(**inputs)
    nc = _get_module(b_loc)
    res = run_bass_kernel_spmd(nc, in_maps, list(range(N_CORES)))
    outs = [np.asarray(res.results[i]["out"]) for i in range(N_CORES)]
    full = np.concatenate(outs, axis=0).reshape(B, C, H, W)
    return full.astype(np.float32)


if __name__ == "__main__":
    rng = np.random.default_rng(0)
    inputs = dict(
        style_feat=rng.standard_normal((B, C, H, W), dtype=np.float32),
        fw1=(rng.standard_normal((C, C), dtype=np.float32) * 0.02),
        fwd_=(rng.standard_normal((C, 1, 3, 3), dtype=np.float32) * 0.02),
        gw1=(rng.standard_normal((C, C), dtype=np.float32) * 0.02),
        gwd=(rng.standard_normal((C, 1, 3, 3), dtype=np.float32) * 0.02),
        hw1=(rng.standard_normal((C, C), dtype=np.float32) * 0.02),
        hwd=(rng.standard_normal((C, 1, 3, 3), dtype=np.float32) * 0.02),
        ow=(rng.standard_normal((C, C), dtype=np.float32) * 0.02),
        temperature=np.ones((HEADS, 1, 1), dtype=np.float32),
    )
    o = kernel(**inputs)
    print(o.shape, o.dtype)


# revision 14
# speedup vs baseline: 1.4694x; 1.0322x over previous
"""Trainium2 Bass kernel for DilatedMDTA (dense_transformer).

Computation (per batch image X [512, 64, 64]):
  q = DW_f(fw1 @ X) ; k = DW_g(gw1 @ X) ; v = DW_h(hw1 @ X)
  where DW_* is a depthwise 3x3 dilation-2 conv with reflection pad 2.
  energy[h] = q_h @ k_h^T  (contract over the 4096 pixels)
  attn = softmax(energy * temperature, axis=-1)
  out = ow @ (attn @ v) + X

Sharding: data-parallel over batch B=16 across 8 cores (2 images/core).

Per-core mapping (v2):
  - input X is reflection-padded on the host and shipped as fp8 (e4m3);
    all four 1x1 convs run as fp8 DoubleRow matmuls (2 k-tiles/pass).
  - v-branch depthwise conv is FOLDED into the 1x1 conv on the PE for
    mts in FOLD_V_MT: 9 shifted-window accumulation passes with host
    prescaled weights W_t = diag(wd[:,t]) @ W (no elementwise tap work).
  - remaining tiles: conv psum is scattered into a padded SBUF buffer
    (ACT, scale fused), taps split DVE (tensor_scalar mul 4x + add 2x)
    and GPSIMD (scalar_tensor_tensor chain).
  - energy per head-pair as one [128]x[128] PSUM accumulation over 32
    pixel chunks of DMA-transposed qT/kT.
  - softmax: plain exp (logits are O(0.1)), 1/rowsum fused into the
    attn@v PSUM evacuation; attnout emitted as fp8 (x256) so the output
    conv also runs DoubleRow.
  - residual: identity*16384 matmul accumulates X into the ow psum
    (scales: wo x64, attnout x256 -> psum = 16384*(conv+X), evac /16384).
  - output written bf16, upcast on host.
"""

import numpy as np
import ml_dtypes

import concourse.bass as bass
from concourse import bacc
import concourse.mybir as mybir
import concourse.tile as tile
from concourse.bass import ts
from concourse.bass_utils import run_bass_kernel_spmd
from concourse.masks import make_identity

BF16 = mybir.dt.bfloat16
F32 = mybir.dt.float32
FP8 = mybir.dt.float8e4
DR = mybir.MatmulPerfMode.DoubleRow
AX = mybir.AxisListType.X
MUL = mybir.AluOpType.mult
ADD = mybir.AluOpType.add
COPY = mybir.ActivationFunctionType.Copy

N_CORES = 8
B = 16
C = 512
H = W = 64
HW = H * W
HEADS = 8
CPH = C // HEADS  # 64
P = 128
NT = C // P      # 4 channel tiles
NCH = 8          # 512-px chunks per image
NW = HW // NCH   # 512
PW = W + 4       # 68 padded width
PAD_SZ = PW * PW

SW = 64.0        # fp8 scale for unfolded 1x1 weights
SFOLD = 2048.0   # fp8 scale for folded (tap-premultiplied) weights
SATT = 256.0     # attnout scale (fused into v evacuation)
SRES = SW * SATT  # net scale of the ow psum (16384)

FOLD_V_MT = (0, 1, 2)   # v-branch mts whose DW is folded into the PE conv
GPS_TAPS = (6, 7, 8)    # taps done on GPSIMD for elementwise tiles
ACT_TAP0_MT = (0, 2)    # mts whose tap-0 product is offloaded to ACT


def _r(ap, spec, **kw):
    return ap.rearrange(spec, **kw)


def build_module(b_loc: int):
    nc = bacc.Bacc("TRN2", target_bir_lowering=False, debug=False)

    xp8 = nc.dram_tensor("xp8", [b_loc, P, NT * PAD_SZ], FP8, kind="ExternalInput").ap()
    xres = nc.dram_tensor("xres", [b_loc, C, HW], BF16, kind="ExternalInput").ap()
    wq = nc.dram_tensor("wq", [P, NT * C], FP8, kind="ExternalInput").ap()
    wk = nc.dram_tensor("wk", [P, NT * C], FP8, kind="ExternalInput").ap()
    wv = nc.dram_tensor("wv", [P, NT * C], FP8, kind="ExternalInput").ap()
    wfold = nc.dram_tensor("wfold", [P, 9 * NT * C], FP8, kind="ExternalInput").ap()
    wo = nc.dram_tensor("wo", [P, NT * C], FP8, kind="ExternalInput").ap()
    wd = nc.dram_tensor("wd", [P, 3 * NT * 9], F32, kind="ExternalInput").ap()
    sq = nc.dram_tensor("sq", [P, NT], F32, kind="ExternalInput").ap()
    out = nc.dram_tensor("out", [b_loc, C, HW], BF16, kind="ExternalOutput").ap()

    with tile.TileContext(nc) as tc:
        _body(tc, b_loc, xp8, xres, [wq, wk, wv], wfold, wo, wd, sq, out)
    nc.compile()
    return nc


def _body(tc, b_loc, xp8, xres, wqkv, wfold, wo, wd, sq, out):
    nc = tc.nc

    pools = []

    def mkpool(**kw):
        p = tc.alloc_tile_pool(**kw)
        pools.append(p)
        return p

    const = mkpool(name="const", bufs=1)
    x8_pool = mkpool(name="x8", bufs=1)
    xr_pool = mkpool(name="xr", bufs=1)
    xpad_pool = mkpool(name="xpad", bufs=3)
    qk_pool = mkpool(name="qk", bufs=2)
    v_pool = mkpool(name="v", bufs=2)
    qt_pool = mkpool(name="qt", bufs=1)
    att_pool = mkpool(name="att", bufs=1)
    small_pool = mkpool(name="small", bufs=2)
    prod_dve = mkpool(name="prodd", bufs=3)
    prod_act = mkpool(name="proda", bufs=1)
    outp = mkpool(name="outp", bufs=2)
    ps_fold = mkpool(name="ps_fold", bufs=2, space="PSUM")
    ps_conv = mkpool(name="ps_conv", bufs=4, space="PSUM")
    ps_e = mkpool(name="ps_e", bufs=1, space="PSUM")
    ps_t = mkpool(name="ps_t", bufs=1, space="PSUM")

    # weights / consts
    w_sb = []
    for name, wdram in zip("qkv", wqkv):
        t = const.tile([P, NT, C], FP8, tag=f"w{name}")
        nc.sync.dma_start(t[:], _r(wdram, "p (kt o) -> p kt o", kt=NT))
        w_sb.append(t)
    wfold_sb = const.tile([P, 9, NT, C], FP8, tag="wfold")
    nc.sync.dma_start(wfold_sb[:], _r(wfold, "p (t kt o) -> p t kt o", t=9, kt=NT))
    wo_sb = const.tile([P, NT, C], FP8, tag="wo")
    nc.sync.dma_start(wo_sb[:], _r(wo, "p (kt o) -> p kt o", kt=NT))
    wd_sb = const.tile([P, 3 * NT * 9], F32, tag="wd")
    nc.sync.dma_start(wd_sb[:], wd[:])
    sq_sb = const.tile([P, NT], F32, tag="sq")
    nc.sync.dma_start(sq_sb[:], sq[:])
    ident = const.tile([P, P], BF16, tag="ident")
    make_identity(nc, ident[:])
    identr = const.tile([P, P], BF16, tag="identr")
    nc.scalar.mul(identr[:], ident[:], float(SRES))

    def conv_scatter(br, mt, x8t, xpad):
        """1x1 conv of the full PADDED input (pointwise: conv(pad(X)) =
        pad(conv(X))) -> padded buffer with no pad copies at all.
        10 chunks of 7 padded rows (last 5), contiguous in and out."""
        scale = sq_sb[:, mt : mt + 1] if br == 0 else (SATT / SW if br == 2 else 1.0 / SW)
        for ch in range(10):
            r0 = 7 * ch
            ncol = (5 if ch == 9 else 7) * PW
            ps = ps_conv.tile([P, NW], F32, tag="cps")
            for i in range(2):
                nc.tensor.matmul(
                    ps[:, 0:ncol],
                    w_sb[br][:, 2 * i : 2 * i + 2, ts(mt, P)],
                    x8t[:, 2 * i : 2 * i + 2, r0 * PW : r0 * PW + ncol],
                    start=(i == 0),
                    stop=(i == 1),
                    perf_mode=DR,
                )
            nc.scalar.activation(
                xpad[:, r0 * PW : r0 * PW + ncol], ps[:, 0:ncol], COPY, scale=scale
            )

    def dw_taps(br, mt, xpv, y):
        """9-tap depthwise: 2 muls on ACT (incl. the y init), the rest
        muls at 4x + all adds at 2x on DVE. GPSIMD stays idle: its ops
        steal the SBUF port DVE needs for 2x/4x modes."""

        def wsc(t):
            i = (br * NT + mt) * 9 + t
            return wd_sb[:, i : i + 1]

        def srcf(t):
            i, j = t // 3, t % 3
            return xpv[:, 2 * i : 2 * i + H, 2 * j : 2 * j + W]

        yv = _r(y[:], "p (r c) -> p r c", c=W)
        # ACT initializes y with tap 0 and makes the tap-1 product
        nc.scalar.activation(yv, srcf(0), COPY, scale=wsc(0))
        pf0 = prod_act.tile([P, HW], BF16, tag="pf0")
        nc.scalar.activation(
            _r(pf0[:], "p (r c) -> p r c", c=W), srcf(1), COPY, scale=wsc(1)
        )
        for t in range(2, 9):
            pf = prod_dve.tile([P, HW], BF16, tag="pf")
            nc.vector.tensor_scalar_mul(_r(pf[:], "p (r c) -> p r c", c=W), srcf(t), wsc(t))
            nc.vector.tensor_add(y[:], y[:], pf[:])
        nc.vector.tensor_add(y[:], y[:], pf0[:])

    def folded_tile(mt, xp8v, y):
        """v-branch conv+DW fused on the PE: 9 shifted-window DR passes.
        LDWEIGHTS is per-matmul anyway, so accumulate one 512-px chunk
        (1 psum bank) at a time to keep PSUM pressure minimal."""
        for ch in range(NCH):
            ps = ps_fold.tile([P, NW], F32, tag="fps")
            r0 = 8 * ch
            for t in range(9):
                ir, jc = t // 3, t % 3
                for i in range(2):
                    nc.tensor.matmul(
                        _r(ps[:], "p (r c) -> p r c", c=W),
                        wfold_sb[:, t, 2 * i : 2 * i + 2, ts(mt, P)],
                        xp8v[:, 2 * i : 2 * i + 2, r0 + 2 * ir : r0 + 2 * ir + 8, 2 * jc : 2 * jc + W],
                        start=(t == 0 and i == 0),
                        stop=(t == 8 and i == 1),
                        perf_mode=DR,
                    )
            nc.scalar.activation(
                y[:, ts(ch, NW)], ps[:], COPY, scale=SATT / SFOLD
            )

    def attention(mt, qT, kT, v, attnout):
        # energy for head pair (2*mt, 2*mt+1); head-cross blocks unused
        eps = ps_e.tile([P, P], F32, tag="eps")
        for nk in range(32):
            nc.tensor.matmul(
                eps[:], qT[:, nk], kT[:, nk], start=(nk == 0), stop=(nk == 31)
            )
        s = small_pool.tile([P, 1], F32, tag="s")
        r = small_pool.tile([P, 1], F32, tag="r")
        exps = small_pool.tile([P, P], BF16, tag="exps")
        # energies here are O(0.1): plain exp is safe, no max subtraction
        nc.scalar.activation(
            exps[:], eps[:], mybir.ActivationFunctionType.Exp, bias=0.0, scale=1.0
        )
        for hh in range(2):
            h0 = CPH * hh
            nc.vector.reduce_sum(
                s[h0 : h0 + CPH], exps[h0 : h0 + CPH, h0 : h0 + CPH], axis=AX
            )
            nc.vector.reciprocal(r[h0 : h0 + CPH], s[h0 : h0 + CPH])

        tps = ps_t.tile([P, P], BF16, tag="tps")
        nc.tensor.transpose(tps[:], exps[:], ident[:])
        attnT = small_pool.tile([P, P], BF16, tag="attnT")
        nc.scalar.copy(attnT[:], tps[:])

        # attn @ v: both heads into one psum bank (concurrent quadrants),
        # single evacuation scaled by 1/rowsum (v carries x256 already)
        for nch in range(NCH):
            pa = ps_conv.tile([P, NW], F32, tag="cps")
            for hh in range(2):
                h0 = CPH * hh
                nc.tensor.matmul(
                    pa[h0 : h0 + CPH],
                    attnT[h0 : h0 + CPH, h0 : h0 + CPH],
                    v[h0 : h0 + CPH, ts(nch, NW)],
                    start=True,
                    stop=True,
                    tile_position=(h0, h0),
                )
            nc.scalar.activation(
                attnout[:, mt, ts(nch, NW)], pa[:], COPY, scale=r[:]
            )

    def ow_block(b, attnout, xrt):
        for mt in range(NT):
            for nch in range(NCH):
                ps = ps_conv.tile([P, NW], F32, tag="cps")
                for i in range(2):
                    nc.tensor.matmul(
                        ps[:],
                        wo_sb[:, 2 * i : 2 * i + 2, ts(mt, P)],
                        attnout[:, 2 * i : 2 * i + 2, ts(nch, NW)],
                        start=(i == 0),
                        stop=False,
                        perf_mode=DR,
                    )
                # residual: += 16384 * X  (identity premultiplied by SRES)
                nc.tensor.matmul(
                    ps[:], identr[:], xrt[:, mt, ts(nch, NW)],
                    start=False, stop=True,
                )
                ot = outp.tile([P, NW], BF16, tag="ot")
                nc.scalar.activation(ot[:], ps[:], COPY, scale=1.0 / SRES)
                nc.sync.dma_start(out[b, ts(mt, P), ts(nch, NW)], ot[:])

    pending_ow = None
    for b in range(b_loc):
        x8t = x8_pool.tile([P, NT, PAD_SZ], FP8, tag="x8t")
        nc.sync.dma_start(_r(x8t[:], "p kt s -> p (kt s)"), xp8[b])
        xp8v = _r(x8t[:], "p kt (r c) -> p kt r c", c=PW)
        xrt = xr_pool.tile([P, NT, HW], BF16, tag="xrt")
        nc.sync.dma_start(xrt[:], _r(xres[b], "(kt p) n -> p kt n", p=P))

        attnout = att_pool.tile([P, NT, HW], FP8, tag="attnout")
        pending = None  # deferred attention block for software pipelining

        for mt in range(NT):
            if mt == 1 and pending_ow is not None:
                ow_block(*pending_ow)
                pending_ow = None
            ydw = {}
            for br in range(3):
                pool = v_pool if br == 2 else qk_pool
                y = pool.tile([P, HW], BF16, tag="v" if br == 2 else "qk")
                if br == 2 and mt in FOLD_V_MT:
                    folded_tile(mt, xp8v, y)
                else:
                    xpad = xpad_pool.tile([P, PAD_SZ], BF16, tag="xpad")
                    xpv = _r(xpad[:], "p (r c) -> p r c", c=PW)
                    # for v the SATT scale rides the conv evacuation (taps
                    # are linear), so y is 256*v in every path
                    conv_scatter(br, mt, x8t, xpad[:])
                    dw_taps(br, mt, xpv, y)
                ydw[br] = y

            qT = qt_pool.tile([P, 32, P], BF16, tag="qT")
            kT = qt_pool.tile([P, 32, P], BF16, tag="kT")
            for qq in range(4):
                nc.sync.dma_start_transpose(
                    qT[:, qq * 8 : (qq + 1) * 8], ydw[0][:, ts(qq, 1024)]
                )
                nc.sync.dma_start_transpose(
                    kT[:, qq * 8 : (qq + 1) * 8], ydw[1][:, ts(qq, 1024)]
                )

            if pending is not None:
                attention(*pending)
            pending = (mt, qT, kT, ydw[2], attnout)

        attention(*pending)
        pending_ow = (b, attnout, xrt)

    ow_block(*pending_ow)

    for p in reversed(pools):
        p.release()


def _fp8(a):
    return np.clip(np.asarray(a, np.float32), -240.0, 240.0).astype(
        ml_dtypes.float8_e4m3
    )


def prep_inputs(style_feat, fw1, fwd_, gw1, gwd, hw1, hwd, ow, temperature):
    """Host-side prep: pad+quantize input, prescale weights, shard over batch."""
    bf16 = ml_dtypes.bfloat16
    sf = np.asarray(style_feat, np.float32)
    temp = np.asarray(temperature, np.float32).reshape(HEADS)

    # padded fp8 input: [B, P, NT*PAD_SZ]
    xpad = np.pad(sf, ((0, 0), (0, 0), (2, 2), (2, 2)), mode="reflect")
    xpad = xpad.reshape(B, NT, P, PAD_SZ).transpose(0, 2, 1, 3).reshape(B, P, NT * PAD_SZ)
    xp8 = _fp8(xpad)

    xres = sf.reshape(B, C, HW).astype(bf16)

    def wT(m, scale):  # [P, NT*C]: [p, kt, o] = m[o, kt*128+p] * scale
        a = (np.asarray(m, np.float32).T * scale).reshape(NT, P, C)
        return a.transpose(1, 0, 2).reshape(P, NT * C)

    wq8 = _fp8(wT(fw1, SW))
    wk8 = _fp8(wT(gw1, SW))
    wv8 = _fp8(wT(hw1, SW))
    wo8 = _fp8(wT(ow, SW))

    # folded v weights: [p, t, kt, o] = hwd[o,t] * hw1[o, kt*128+p] * SFOLD
    wd_v = np.asarray(hwd, np.float32).reshape(C, 9)
    m = np.asarray(hw1, np.float32)
    a = np.einsum("ot,ok->tko", wd_v, m) * SFOLD  # [9, C_in, C_out]
    a = a.reshape(9, NT, P, C).transpose(2, 0, 1, 3).reshape(P, 9 * NT * C)
    wfold8 = _fp8(a)

    # depthwise tap weights -> [128, branch*ctile*9]
    wd_all = np.zeros((P, 3 * NT * 9), dtype=np.float32)
    for bi, wdb in enumerate([fwd_, gwd, hwd]):
        wdb = np.asarray(wdb, np.float32).reshape(C, 9)
        for mt in range(NT):
            wd_all[:, (bi * NT + mt) * 9 : (bi * NT + mt) * 9 + 9] = wdb[
                mt * P : (mt + 1) * P
            ]

    # q evacuation scale: temp per output channel / SW
    tvec = np.repeat(temp, CPH)  # [C]
    sq = (tvec / SW).reshape(NT, P).T.copy()  # [P, NT]

    b_loc = B // N_CORES
    in_maps = []
    for ci in range(N_CORES):
        sl = slice(ci * b_loc, (ci + 1) * b_loc)
        in_maps.append(
            dict(
                xp8=np.ascontiguousarray(xp8[sl]),
                xres=np.ascontiguousarray(xres[sl]),
                wq=wq8, wk=wk8, wv=wv8, wfold=wfold8, wo=wo8,
                wd=wd_all, sq=np.ascontiguousarray(sq),
            )
        )
    return in_maps, b_loc


_CACHED = {}


def _get_module(b_loc):
    if b_loc not in _CACHED:
        _CACHED[b_loc] = build_module(b_loc)
    return _CACHED[b_loc]


def kernel(**inputs):
    in_maps, b_loc = prep_inputs(**inputs)
    nc = _get_module(b_loc)
    res = run_bass_kernel_spmd(nc, in_maps, list(range(N_CORES)))
    outs = [np.asarray(res.results[i]["out"]) for i in range(N_CORES)]
    full = np.concatenate(outs, axis=0).reshape(B, C, H, W)
    return full.astype(np.float32)


if __name__ == "__main__":
    rng = np.random.default_rng(0)
    inputs = dict(
        style_feat=rng.standard_normal((B, C, H, W), dtype=np.float32),
        fw1=(rng.standard_normal((C, C), dtype=np.float32) * 0.02),
        fwd_=(rng.standard_normal((C, 1, 3, 3), dtype=np.float32) * 0.02),
        gw1=(rng.standard_normal((C, C), dtype=np.float32) * 0.02),
        gwd=(rng.standard_normal((C, 1, 3, 3), dtype=np.float32) * 0.02),
        hw1=(rng.standard_normal((C, C), dtype=np.float32) * 0.02),
        hwd=(rng.standard_normal((C, 1, 3, 3), dtype=np.float32) * 0.02),
        ow=(rng.standard_normal((C, C), dtype=np.float32) * 0.02),
        temperature=np.ones((HEADS, 1, 1), dtype=np.float32),
    )
    o = kernel(**inputs)
    print(o.shape, o.dtype)


# revision 17
# speedup vs baseline: 1.5054x; 1.0245x over previous
"""Trainium2 Bass kernel for DilatedMDTA (dense_transformer).

Computation (per batch image X [512, 64, 64]):
  q = DW_f(fw1 @ X) ; k = DW_g(gw1 @ X) ; v = DW_h(hw1 @ X)
  where DW_* is a depthwise 3x3 dilation-2 conv with reflection pad 2.
  energy[h] = q_h @ k_h^T  (contract over the 4096 pixels)
  attn = softmax(energy * temperature, axis=-1)
  out = ow @ (attn @ v) + X

Sharding: data-parallel over batch B=16 across 8 cores (2 images/core).

Per-core mapping (v2):
  - input X is reflection-padded on the host and shipped as fp8 (e4m3);
    all four 1x1 convs run as fp8 DoubleRow matmuls (2 k-tiles/pass).
  - v-branch depthwise conv is FOLDED into the 1x1 conv on the PE for
    mts in FOLD_V_MT: 9 shifted-window accumulation passes with host
    prescaled weights W_t = diag(wd[:,t]) @ W (no elementwise tap work).
  - remaining tiles: conv psum is scattered into a padded SBUF buffer
    (ACT, scale fused), taps split DVE (tensor_scalar mul 4x + add 2x)
    and GPSIMD (scalar_tensor_tensor chain).
  - energy per head-pair as one [128]x[128] PSUM accumulation over 32
    pixel chunks of DMA-transposed qT/kT.
  - softmax: plain exp (logits are O(0.1)), 1/rowsum fused into the
    attn@v PSUM evacuation; attnout emitted as fp8 (x256) so the output
    conv also runs DoubleRow.
  - residual: identity*16384 matmul accumulates X into the ow psum
    (scales: wo x64, attnout x256 -> psum = 16384*(conv+X), evac /16384).
  - output written bf16, upcast on host.
"""

import numpy as np
import ml_dtypes

import concourse.bass as bass
from concourse import bacc
import concourse.mybir as mybir
import concourse.tile as tile
from concourse.bass import ts
from concourse.bass_utils import run_bass_kernel_spmd
from concourse.masks import make_identity

BF16 = mybir.dt.bfloat16
F32 = mybir.dt.float32
FP8 = mybir.dt.float8e4
DR = mybir.MatmulPerfMode.DoubleRow
AX = mybir.AxisListType.X
MUL = mybir.AluOpType.mult
ADD = mybir.AluOpType.add
COPY = mybir.ActivationFunctionType.Copy

N_CORES = 8
B = 16
C = 512
H = W = 64
HW = H * W
HEADS = 8
CPH = C // HEADS  # 64
P = 128
NT = C // P      # 4 channel tiles
NCH = 8          # 512-px chunks per image
NW = HW // NCH   # 512
PW = W + 4       # 68 padded width
PAD_SZ = PW * PW

SW = 64.0        # fp8 scale for unfolded 1x1 weights
SFOLD = 2048.0   # fp8 scale for folded (tap-premultiplied) weights
SATT = 256.0     # attnout scale (fused into v evacuation)
SRES = SW * SATT  # net scale of the ow psum (16384)

FOLD_V_MT = (0, 1, 2)   # v-branch mts whose DW is folded into the PE conv
GPS_TAPS = (6, 7, 8)    # taps done on GPSIMD for elementwise tiles
ACT_TAP0_MT = (0, 2)    # mts whose tap-0 product is offloaded to ACT


def _r(ap, spec, **kw):
    return ap.rearrange(spec, **kw)


def build_module(b_loc: int):
    nc = bacc.Bacc("TRN2", target_bir_lowering=False, debug=False)

    xp8 = nc.dram_tensor("xp8", [b_loc, P, NT * PAD_SZ], FP8, kind="ExternalInput").ap()
    xres = nc.dram_tensor("xres", [b_loc, C, HW], BF16, kind="ExternalInput").ap()
    wq = nc.dram_tensor("wq", [P, NT * C], FP8, kind="ExternalInput").ap()
    wk = nc.dram_tensor("wk", [P, NT * C], FP8, kind="ExternalInput").ap()
    wv = nc.dram_tensor("wv", [P, NT * C], FP8, kind="ExternalInput").ap()
    wfold = nc.dram_tensor("wfold", [P, 9 * NT * C], FP8, kind="ExternalInput").ap()
    wo = nc.dram_tensor("wo", [P, NT * C], FP8, kind="ExternalInput").ap()
    wd = nc.dram_tensor("wd", [P, 3 * NT * 9], F32, kind="ExternalInput").ap()
    sq = nc.dram_tensor("sq", [P, NT], F32, kind="ExternalInput").ap()
    out = nc.dram_tensor("out", [b_loc, C, HW], BF16, kind="ExternalOutput").ap()

    with tile.TileContext(nc) as tc:
        _body(tc, b_loc, xp8, xres, [wq, wk, wv], wfold, wo, wd, sq, out)
    nc.compile()
    return nc


def _body(tc, b_loc, xp8, xres, wqkv, wfold, wo, wd, sq, out):
    nc = tc.nc

    pools = []

    def mkpool(**kw):
        p = tc.alloc_tile_pool(**kw)
        pools.append(p)
        return p

    const = mkpool(name="const", bufs=1)
    x8_pool = mkpool(name="x8", bufs=2)
    xr_pool = mkpool(name="xr", bufs=1)
    xpad_pool = mkpool(name="xpad", bufs=2)
    qk_pool = mkpool(name="qk", bufs=2)
    v_pool = mkpool(name="v", bufs=2)
    qt_pool = mkpool(name="qt", bufs=1)
    att_pool = mkpool(name="att", bufs=1)
    small_pool = mkpool(name="small", bufs=2)
    prod_dve = mkpool(name="prodd", bufs=2)
    prod_act = mkpool(name="proda", bufs=1)
    outp = mkpool(name="outp", bufs=2)
    ps_fold = mkpool(name="ps_fold", bufs=2, space="PSUM")
    ps_conv = mkpool(name="ps_conv", bufs=4, space="PSUM")
    ps_e = mkpool(name="ps_e", bufs=1, space="PSUM")
    ps_t = mkpool(name="ps_t", bufs=1, space="PSUM")

    # weights / consts
    w_sb = []
    for name, wdram in zip("qkv", wqkv):
        t = const.tile([P, NT, C], FP8, tag=f"w{name}")
        nc.gpsimd.dma_start(t[:], _r(wdram, "p (kt o) -> p kt o", kt=NT))
        w_sb.append(t)
    wfold_sb = const.tile([P, 9, NT, C], FP8, tag="wfold")
    nc.gpsimd.dma_start(wfold_sb[:], _r(wfold, "p (t kt o) -> p t kt o", t=9, kt=NT))
    wo_sb = const.tile([P, NT, C], FP8, tag="wo")
    nc.gpsimd.dma_start(wo_sb[:], _r(wo, "p (kt o) -> p kt o", kt=NT))
    wd_sb = const.tile([P, 3 * NT * 9], F32, tag="wd")
    nc.gpsimd.dma_start(wd_sb[:], wd[:])
    sq_sb = const.tile([P, NT], F32, tag="sq")
    nc.gpsimd.dma_start(sq_sb[:], sq[:])
    ident = const.tile([P, P], BF16, tag="ident")
    make_identity(nc, ident[:])
    identr = const.tile([P, P], BF16, tag="identr")
    nc.scalar.mul(identr[:], ident[:], float(SRES))

    def conv_scatter(br, mt, x8t, xpad):
        """1x1 conv of the full PADDED input (pointwise: conv(pad(X)) =
        pad(conv(X))) -> padded buffer with no pad copies at all.
        10 chunks of 7 padded rows (last 5), contiguous in and out."""
        scale = sq_sb[:, mt : mt + 1] if br == 0 else (SATT / SW if br == 2 else 1.0 / SW)
        for ch in range(10):
            r0 = 7 * ch
            ncol = (5 if ch == 9 else 7) * PW
            ps = ps_conv.tile([P, NW], F32, tag="cps")
            for i in range(2):
                nc.tensor.matmul(
                    ps[:, 0:ncol],
                    w_sb[br][:, 2 * i : 2 * i + 2, ts(mt, P)],
                    x8t[:, 2 * i : 2 * i + 2, r0 * PW : r0 * PW + ncol],
                    start=(i == 0),
                    stop=(i == 1),
                    perf_mode=DR,
                )
            nc.scalar.activation(
                xpad[:, r0 * PW : r0 * PW + ncol], ps[:, 0:ncol], COPY, scale=scale
            )

    def dw_taps(br, mt, xpv, y):
        """9-tap depthwise: 2 muls on ACT (incl. the y init), the rest
        muls at 4x + all adds at 2x on DVE. GPSIMD stays idle: its ops
        steal the SBUF port DVE needs for 2x/4x modes."""

        def wsc(t):
            i = (br * NT + mt) * 9 + t
            return wd_sb[:, i : i + 1]

        def srcf(t):
            i, j = t // 3, t % 3
            return xpv[:, 2 * i : 2 * i + H, 2 * j : 2 * j + W]

        yv = _r(y[:], "p (r c) -> p r c", c=W)
        # ACT initializes y with tap 0 and makes the tap-1 product
        nc.scalar.activation(yv, srcf(0), COPY, scale=wsc(0))
        pf0 = prod_act.tile([P, HW], BF16, tag="pf0")
        nc.scalar.activation(
            _r(pf0[:], "p (r c) -> p r c", c=W), srcf(1), COPY, scale=wsc(1)
        )
        for t in range(2, 9):
            pf = prod_dve.tile([P, HW], BF16, tag="pf")
            nc.vector.tensor_scalar_mul(_r(pf[:], "p (r c) -> p r c", c=W), srcf(t), wsc(t))
            nc.vector.tensor_add(y[:], y[:], pf[:])
        nc.vector.tensor_add(y[:], y[:], pf0[:])

    def folded_tile(mt, xp8v, y):
        """v-branch conv+DW fused on the PE: 9 shifted-window DR passes.
        LDWEIGHTS is per-matmul anyway, so accumulate one 512-px chunk
        (1 psum bank) at a time to keep PSUM pressure minimal."""
        for ch in range(NCH):
            ps = ps_fold.tile([P, NW], F32, tag="fps")
            r0 = 8 * ch
            for t in range(9):
                ir, jc = t // 3, t % 3
                for i in range(2):
                    nc.tensor.matmul(
                        _r(ps[:], "p (r c) -> p r c", c=W),
                        wfold_sb[:, t, 2 * i : 2 * i + 2, ts(mt, P)],
                        xp8v[:, 2 * i : 2 * i + 2, r0 + 2 * ir : r0 + 2 * ir + 8, 2 * jc : 2 * jc + W],
                        start=(t == 0 and i == 0),
                        stop=(t == 8 and i == 1),
                        perf_mode=DR,
                    )
            nc.scalar.activation(
                y[:, ts(ch, NW)], ps[:], COPY, scale=SATT / SFOLD
            )

    def attention(mt, qT, kT, v, attnout):
        # energy for head pair (2*mt, 2*mt+1); head-cross blocks unused
        eps = ps_e.tile([P, P], F32, tag="eps")
        for nk in range(32):
            nc.tensor.matmul(
                eps[:], qT[:, nk], kT[:, nk], start=(nk == 0), stop=(nk == 31)
            )
        s = small_pool.tile([P, 1], F32, tag="s")
        r = small_pool.tile([P, 1], F32, tag="r")
        exps = small_pool.tile([P, P], BF16, tag="exps")
        # energies here are O(0.1): plain exp is safe, no max subtraction
        nc.scalar.activation(
            exps[:], eps[:], mybir.ActivationFunctionType.Exp, bias=0.0, scale=1.0
        )
        for hh in range(2):
            h0 = CPH * hh
            nc.vector.reduce_sum(
                s[h0 : h0 + CPH], exps[h0 : h0 + CPH, h0 : h0 + CPH], axis=AX
            )
            nc.vector.reciprocal(r[h0 : h0 + CPH], s[h0 : h0 + CPH])

        tps = ps_t.tile([P, P], BF16, tag="tps")
        nc.tensor.transpose(tps[:], exps[:], ident[:])
        attnT = small_pool.tile([P, P], BF16, tag="attnT")
        nc.scalar.copy(attnT[:], tps[:])

        # attn @ v: both heads into one psum bank (concurrent quadrants),
        # single evacuation scaled by 1/rowsum (v carries x256 already)
        for nch in range(NCH):
            pa = ps_conv.tile([P, NW], F32, tag="cps")
            for hh in range(2):
                h0 = CPH * hh
                nc.tensor.matmul(
                    pa[h0 : h0 + CPH],
                    attnT[h0 : h0 + CPH, h0 : h0 + CPH],
                    v[h0 : h0 + CPH, ts(nch, NW)],
                    start=True,
                    stop=True,
                    tile_position=(h0, h0),
                )
            nc.scalar.activation(
                attnout[:, mt, ts(nch, NW)], pa[:], COPY, scale=r[:]
            )

    def ow_block(b, attnout, xrt):
        for mt in range(NT):
            for nch in range(NCH):
                ps = ps_conv.tile([P, NW], F32, tag="cps")
                for i in range(2):
                    nc.tensor.matmul(
                        ps[:],
                        wo_sb[:, 2 * i : 2 * i + 2, ts(mt, P)],
                        attnout[:, 2 * i : 2 * i + 2, ts(nch, NW)],
                        start=(i == 0),
                        stop=False,
                        perf_mode=DR,
                    )
                # residual: += 16384 * X  (identity premultiplied by SRES)
                nc.tensor.matmul(
                    ps[:], identr[:], xrt[:, mt, ts(nch, NW)],
                    start=False, stop=True,
                )
                ot = outp.tile([P, NW], BF16, tag="ot")
                nc.scalar.activation(ot[:], ps[:], COPY, scale=1.0 / SRES)
                nc.gpsimd.dma_start(out[b, ts(mt, P), ts(nch, NW)], ot[:])

    pending_ow = None
    for b in range(b_loc):
        x8t = x8_pool.tile([P, NT, PAD_SZ], FP8, tag="x8t")
        nc.gpsimd.dma_start(_r(x8t[:], "p kt s -> p (kt s)"), xp8[b])
        xp8v = _r(x8t[:], "p kt (r c) -> p kt r c", c=PW)
        xrt = xr_pool.tile([P, NT, HW], BF16, tag="xrt")
        nc.gpsimd.dma_start(xrt[:], _r(xres[b], "(kt p) n -> p kt n", p=P))

        attnout = att_pool.tile([P, NT, HW], FP8, tag="attnout")
        pending = None  # deferred attention block for software pipelining

        for mt in range(NT):
            if mt == 1 and pending_ow is not None:
                ow_block(*pending_ow)
                pending_ow = None
            ydw = {}
            for br in range(3):
                pool = v_pool if br == 2 else qk_pool
                y = pool.tile([P, HW], BF16, tag="v" if br == 2 else "qk")
                if br == 2 and mt in FOLD_V_MT:
                    folded_tile(mt, xp8v, y)
                else:
                    xpad = xpad_pool.tile([P, PAD_SZ], BF16, tag="xpad")
                    xpv = _r(xpad[:], "p (r c) -> p r c", c=PW)
                    # for v the SATT scale rides the conv evacuation (taps
                    # are linear), so y is 256*v in every path
                    conv_scatter(br, mt, x8t, xpad[:])
                    dw_taps(br, mt, xpv, y)
                ydw[br] = y

            qT = qt_pool.tile([P, 32, P], BF16, tag="qT")
            kT = qt_pool.tile([P, 32, P], BF16, tag="kT")
            for qq in range(2):
                nc.sync.dma_start_transpose(
                    qT[:, qq * 16 : (qq + 1) * 16], ydw[0][:, ts(qq, 2048)]
                )
                nc.sync.dma_start_transpose(
                    kT[:, qq * 16 : (qq + 1) * 16], ydw[1][:, ts(qq, 2048)]
                )

            if pending is not None:
                attention(*pending)
            pending = (mt, qT, kT, ydw[2], attnout)

        attention(*pending)
        pending_ow = (b, attnout, xrt)

    ow_block(*pending_ow)

    for p in reversed(pools):
        p.release()


def _fp8(a):
    return np.clip(np.asarray(a, np.float32), -240.0, 240.0).astype(
        ml_dtypes.float8_e4m3
    )


def prep_inputs(style_feat, fw1, fwd_, gw1, gwd, hw1, hwd, ow, temperature):
    """Host-side prep: pad+quantize input, prescale weights, shard over batch."""
    bf16 = ml_dtypes.bfloat16
    sf = np.asarray(style_feat, np.float32)
    temp = np.asarray(temperature, np.float32).reshape(HEADS)

    # padded fp8 input: [B, P, NT*PAD_SZ]
    xpad = np.pad(sf, ((0, 0), (0, 0), (2, 2), (2, 2)), mode="reflect")
    xpad = xpad.reshape(B, NT, P, PAD_SZ).transpose(0, 2, 1, 3).reshape(B, P, NT * PAD_SZ)
    xp8 = _fp8(xpad)

    xres = sf.reshape(B, C, HW).astype(bf16)

    def wT(m, scale):  # [P, NT*C]: [p, kt, o] = m[o, kt*128+p] * scale
        a = (np.asarray(m, np.float32).T * scale).reshape(NT, P, C)
        return a.transpose(1, 0, 2).reshape(P, NT * C)

    wq8 = _fp8(wT(fw1, SW))
    wk8 = _fp8(wT(gw1, SW))
    wv8 = _fp8(wT(hw1, SW))
    wo8 = _fp8(wT(ow, SW))

    # folded v weights: [p, t, kt, o] = hwd[o,t] * hw1[o, kt*128+p] * SFOLD
    wd_v = np.asarray(hwd, np.float32).reshape(C, 9)
    m = np.asarray(hw1, np.float32)
    a = np.einsum("ot,ok->tko", wd_v, m) * SFOLD  # [9, C_in, C_out]
    a = a.reshape(9, NT, P, C).transpose(2, 0, 1, 3).reshape(P, 9 * NT * C)
    wfold8 = _fp8(a)

    # depthwise tap weights -> [128, branch*ctile*9]
    wd_all = np.zeros((P, 3 * NT * 9), dtype=np.float32)
    for bi, wdb in enumerate([fwd_, gwd, hwd]):
        wdb = np.asarray(wdb, np.float32).reshape(C, 9)
        for mt in range(NT):
            wd_all[:, (bi * NT + mt) * 9 : (bi * NT + mt) * 9 + 9] = wdb[
                mt * P : (mt + 1) * P
            ]

    # q evacuation scale: temp per output channel / SW
    tvec = np.repeat(temp, CPH)  # [C]
    sq = (tvec / SW).reshape(NT, P).T.copy()  # [P, NT]

    b_loc = B // N_CORES
    in_maps = []
    for ci in range(N_CORES):
        sl = slice(ci * b_loc, (ci + 1) * b_loc)
        in_maps.append(
            dict(
                xp8=np.ascontiguousarray(xp8[sl]),
                xres=np.ascontiguousarray(xres[sl]),
                wq=wq8, wk=wk8, wv=wv8, wfold=wfold8, wo=wo8,
                wd=wd_all, sq=np.ascontiguousarray(sq),
            )
        )
    return in_maps, b_loc


_CACHED = {}


def _get_module(b_loc):
    if b_loc not in _CACHED:
        _CACHED[b_loc] = build_module(b_loc)
    return _CACHED[b_loc]


def kernel(**inputs):
    in_maps, b_loc = prep_inputs(**inputs)
    nc = _get_module(b_loc)
    res = run_bass_kernel_spmd(nc, in_maps, list(range(N_CORES)))
    outs = [np.asarray(res.results[i]["out"]) for i in range(N_CORES)]
    full = np.concatenate(outs, axis=0).reshape(B, C, H, W)
    return full.astype(np.float32)


if __name__ == "__main__":
    rng = np.random.default_rng(0)
    inputs = dict(
        style_feat=rng.standard_normal((B, C, H, W), dtype=np.float32),
        fw1=(rng.standard_normal((C, C), dtype=np.float32) * 0.02),
        fwd_=(rng.standard_normal((C, 1, 3, 3), dtype=np.float32) * 0.02),
        gw1=(rng.standard_normal((C, C), dtype=np.float32) * 0.02),
        gwd=(rng.standard_normal((C, 1, 3, 3), dtype=np.float32) * 0.02),
        hw1=(rng.standard_normal((C, C), dtype=np.float32) * 0.02),
        hwd=(rng.standard_normal((C, 1, 3, 3), dtype=np.float32) * 0.02),
        ow=(rng.standard_normal((C, C), dtype=np.float32) * 0.02),
        temperature=np.ones((HEADS, 1, 1), dtype=np.float32),
    )
    o = kernel(**inputs)
    print(o.shape, o.dtype)


# revision 18
# speedup vs baseline: 1.5382x; 1.0218x over previous
"""Trainium2 Bass kernel for DilatedMDTA (dense_transformer).

Computation (per batch image X [512, 64, 64]):
  q = DW_f(fw1 @ X) ; k = DW_g(gw1 @ X) ; v = DW_h(hw1 @ X)
  where DW_* is a depthwise 3x3 dilation-2 conv with reflection pad 2.
  energy[h] = q_h @ k_h^T  (contract over the 4096 pixels)
  attn = softmax(energy * temperature, axis=-1)
  out = ow @ (attn @ v) + X

Sharding: data-parallel over batch B=16 across 8 cores (2 images/core).

Per-core mapping (v2):
  - input X is reflection-padded on the host and shipped as fp8 (e4m3);
    all four 1x1 convs run as fp8 DoubleRow matmuls (2 k-tiles/pass).
  - v-branch depthwise conv is FOLDED into the 1x1 conv on the PE for
    mts in FOLD_V_MT: 9 shifted-window accumulation passes with host
    prescaled weights W_t = diag(wd[:,t]) @ W (no elementwise tap work).
  - remaining tiles: conv psum is scattered into a padded SBUF buffer
    (ACT, scale fused), taps split DVE (tensor_scalar mul 4x + add 2x)
    and GPSIMD (scalar_tensor_tensor chain).
  - energy per head-pair as one [128]x[128] PSUM accumulation over 32
    pixel chunks of DMA-transposed qT/kT.
  - softmax: plain exp (logits are O(0.1)), 1/rowsum fused into the
    attn@v PSUM evacuation; attnout emitted as fp8 (x256) so the output
    conv also runs DoubleRow.
  - residual: identity*16384 matmul accumulates X into the ow psum
    (scales: wo x64, attnout x256 -> psum = 16384*(conv+X), evac /16384).
  - output written bf16, upcast on host.
"""

import numpy as np
import ml_dtypes

import concourse.bass as bass
from concourse import bacc
import concourse.mybir as mybir
import concourse.tile as tile
from concourse.bass import ts
from concourse.bass_utils import run_bass_kernel_spmd
from concourse.masks import make_identity

BF16 = mybir.dt.bfloat16
F32 = mybir.dt.float32
FP8 = mybir.dt.float8e4
DR = mybir.MatmulPerfMode.DoubleRow
AX = mybir.AxisListType.X
MUL = mybir.AluOpType.mult
ADD = mybir.AluOpType.add
COPY = mybir.ActivationFunctionType.Copy

N_CORES = 8
B = 16
C = 512
H = W = 64
HW = H * W
HEADS = 8
CPH = C // HEADS  # 64
P = 128
NT = C // P      # 4 channel tiles
NCH = 8          # 512-px chunks per image
NW = HW // NCH   # 512
PW = W + 4       # 68 padded width
PAD_SZ = PW * PW

SW = 64.0        # fp8 scale for unfolded 1x1 weights
SFOLD = 2048.0   # fp8 scale for folded (tap-premultiplied) weights
SATT = 256.0     # attnout scale (fused into v evacuation)
SRES = SW * SATT  # net scale of the ow psum (16384)

FOLD_V_MT = (0, 1, 2)   # v-branch mts whose DW is folded into the PE conv
GPS_TAPS = (6, 7, 8)    # taps done on GPSIMD for elementwise tiles
ACT_TAP0_MT = (0, 2)    # mts whose tap-0 product is offloaded to ACT


def _r(ap, spec, **kw):
    return ap.rearrange(spec, **kw)


def build_module(b_loc: int):
    nc = bacc.Bacc("TRN2", target_bir_lowering=False, debug=False)

    xp8 = nc.dram_tensor("xp8", [b_loc, P, NT * PAD_SZ], FP8, kind="ExternalInput").ap()
    xres = nc.dram_tensor("xres", [b_loc, C, HW], BF16, kind="ExternalInput").ap()
    wq = nc.dram_tensor("wq", [P, NT * C], FP8, kind="ExternalInput").ap()
    wk = nc.dram_tensor("wk", [P, NT * C], FP8, kind="ExternalInput").ap()
    wv = nc.dram_tensor("wv", [P, NT * C], FP8, kind="ExternalInput").ap()
    wfold = nc.dram_tensor("wfold", [P, 9 * NT * C], FP8, kind="ExternalInput").ap()
    wo = nc.dram_tensor("wo", [P, NT * C], FP8, kind="ExternalInput").ap()
    wd = nc.dram_tensor("wd", [P, 3 * NT * 9], F32, kind="ExternalInput").ap()
    sq = nc.dram_tensor("sq", [P, NT], F32, kind="ExternalInput").ap()
    out = nc.dram_tensor("out", [b_loc, C, HW], BF16, kind="ExternalOutput").ap()

    with tile.TileContext(nc) as tc:
        _body(tc, b_loc, xp8, xres, [wq, wk, wv], wfold, wo, wd, sq, out)
    nc.compile()
    return nc


def _body(tc, b_loc, xp8, xres, wqkv, wfold, wo, wd, sq, out):
    nc = tc.nc

    pools = []

    def mkpool(**kw):
        p = tc.alloc_tile_pool(**kw)
        pools.append(p)
        return p

    const = mkpool(name="const", bufs=1)
    x8_pool = mkpool(name="x8", bufs=1)
    xr_pool = mkpool(name="xr", bufs=1)
    xpad_pool = mkpool(name="xpad", bufs=2)
    qk_pool = mkpool(name="qk", bufs=2)
    v_pool = mkpool(name="v", bufs=2)
    qt_pool = mkpool(name="qt", bufs=1)
    att_pool = mkpool(name="att", bufs=2)
    small_pool = mkpool(name="small", bufs=2)
    prod_dve = mkpool(name="prodd", bufs=2)
    prod_act = mkpool(name="proda", bufs=1)
    outp = mkpool(name="outp", bufs=2)
    ps_fold = mkpool(name="ps_fold", bufs=2, space="PSUM")
    ps_conv = mkpool(name="ps_conv", bufs=4, space="PSUM")
    ps_e = mkpool(name="ps_e", bufs=1, space="PSUM")
    ps_t = mkpool(name="ps_t", bufs=1, space="PSUM")

    # weights / consts
    w_sb = []
    for name, wdram in zip("qkv", wqkv):
        t = const.tile([P, NT, C], FP8, tag=f"w{name}")
        nc.gpsimd.dma_start(t[:], _r(wdram, "p (kt o) -> p kt o", kt=NT))
        w_sb.append(t)
    wfold_sb = const.tile([P, 9, NT, C], FP8, tag="wfold")
    nc.gpsimd.dma_start(wfold_sb[:], _r(wfold, "p (t kt o) -> p t kt o", t=9, kt=NT))
    wo_sb = const.tile([P, NT, C], FP8, tag="wo")
    nc.gpsimd.dma_start(wo_sb[:], _r(wo, "p (kt o) -> p kt o", kt=NT))
    wd_sb = const.tile([P, 3 * NT * 9], F32, tag="wd")
    nc.gpsimd.dma_start(wd_sb[:], wd[:])
    sq_sb = const.tile([P, NT], F32, tag="sq")
    nc.gpsimd.dma_start(sq_sb[:], sq[:])
    ident = const.tile([P, P], BF16, tag="ident")
    make_identity(nc, ident[:])
    identr = const.tile([P, P], BF16, tag="identr")
    nc.scalar.mul(identr[:], ident[:], float(SRES))

    def conv_scatter(br, mt, x8t, xpad):
        """1x1 conv of the full PADDED input (pointwise: conv(pad(X)) =
        pad(conv(X))) -> padded buffer with no pad copies at all.
        10 chunks of 7 padded rows (last 5), contiguous in and out."""
        scale = sq_sb[:, mt : mt + 1] if br == 0 else (SATT / SW if br == 2 else 1.0 / SW)
        for ch in range(10):
            r0 = 7 * ch
            ncol = (5 if ch == 9 else 7) * PW
            ps = ps_conv.tile([P, NW], F32, tag="cps")
            for i in range(2):
                nc.tensor.matmul(
                    ps[:, 0:ncol],
                    w_sb[br][:, 2 * i : 2 * i + 2, ts(mt, P)],
                    x8t[:, 2 * i : 2 * i + 2, r0 * PW : r0 * PW + ncol],
                    start=(i == 0),
                    stop=(i == 1),
                    perf_mode=DR,
                )
            nc.scalar.activation(
                xpad[:, r0 * PW : r0 * PW + ncol], ps[:, 0:ncol], COPY, scale=scale
            )

    def dw_taps(br, mt, xpv, y):
        """9-tap depthwise: 2 muls on ACT (incl. the y init), the rest
        muls at 4x + all adds at 2x on DVE. GPSIMD stays idle: its ops
        steal the SBUF port DVE needs for 2x/4x modes."""

        def wsc(t):
            i = (br * NT + mt) * 9 + t
            return wd_sb[:, i : i + 1]

        def srcf(t):
            i, j = t // 3, t % 3
            return xpv[:, 2 * i : 2 * i + H, 2 * j : 2 * j + W]

        yv = _r(y[:], "p (r c) -> p r c", c=W)
        # ACT initializes y with tap 0 and makes the tap-1 product
        nc.scalar.activation(yv, srcf(0), COPY, scale=wsc(0))
        pf0 = prod_act.tile([P, HW], BF16, tag="pf0")
        nc.scalar.activation(
            _r(pf0[:], "p (r c) -> p r c", c=W), srcf(1), COPY, scale=wsc(1)
        )
        for t in range(2, 9):
            pf = prod_dve.tile([P, HW], BF16, tag="pf")
            nc.vector.tensor_scalar_mul(_r(pf[:], "p (r c) -> p r c", c=W), srcf(t), wsc(t))
            nc.vector.tensor_add(y[:], y[:], pf[:])
        nc.vector.tensor_add(y[:], y[:], pf0[:])

    def folded_tile(mt, xp8v, y):
        """v-branch conv+DW fused on the PE: 9 shifted-window DR passes.
        LDWEIGHTS is per-matmul anyway, so accumulate one 512-px chunk
        (1 psum bank) at a time to keep PSUM pressure minimal."""
        for ch in range(NCH):
            ps = ps_fold.tile([P, NW], F32, tag="fps")
            r0 = 8 * ch
            for t in range(9):
                ir, jc = t // 3, t % 3
                for i in range(2):
                    nc.tensor.matmul(
                        _r(ps[:], "p (r c) -> p r c", c=W),
                        wfold_sb[:, t, 2 * i : 2 * i + 2, ts(mt, P)],
                        xp8v[:, 2 * i : 2 * i + 2, r0 + 2 * ir : r0 + 2 * ir + 8, 2 * jc : 2 * jc + W],
                        start=(t == 0 and i == 0),
                        stop=(t == 8 and i == 1),
                        perf_mode=DR,
                    )
            nc.scalar.activation(
                y[:, ts(ch, NW)], ps[:], COPY, scale=SATT / SFOLD
            )

    def attention(mt, qT, kT, v, attnout):
        # energy for head pair (2*mt, 2*mt+1); head-cross blocks unused
        eps = ps_e.tile([P, P], F32, tag="eps")
        for nk in range(32):
            nc.tensor.matmul(
                eps[:], qT[:, nk], kT[:, nk], start=(nk == 0), stop=(nk == 31)
            )
        s = small_pool.tile([P, 1], F32, tag="s")
        r = small_pool.tile([P, 1], F32, tag="r")
        exps = small_pool.tile([P, P], BF16, tag="exps")
        # energies here are O(0.1): plain exp is safe, no max subtraction
        nc.scalar.activation(
            exps[:], eps[:], mybir.ActivationFunctionType.Exp, bias=0.0, scale=1.0
        )
        for hh in range(2):
            h0 = CPH * hh
            nc.vector.reduce_sum(
                s[h0 : h0 + CPH], exps[h0 : h0 + CPH, h0 : h0 + CPH], axis=AX
            )
            nc.vector.reciprocal(r[h0 : h0 + CPH], s[h0 : h0 + CPH])

        tps = ps_t.tile([P, P], BF16, tag="tps")
        nc.tensor.transpose(tps[:], exps[:], ident[:])
        attnT = small_pool.tile([P, P], BF16, tag="attnT")
        nc.scalar.copy(attnT[:], tps[:])

        # attn @ v: both heads into one psum bank (concurrent quadrants),
        # single evacuation scaled by 1/rowsum (v carries x256 already)
        for nch in range(NCH):
            pa = ps_conv.tile([P, NW], F32, tag="cps")
            for hh in range(2):
                h0 = CPH * hh
                nc.tensor.matmul(
                    pa[h0 : h0 + CPH],
                    attnT[h0 : h0 + CPH, h0 : h0 + CPH],
                    v[h0 : h0 + CPH, ts(nch, NW)],
                    start=True,
                    stop=True,
                    tile_position=(h0, h0),
                )
            nc.scalar.activation(
                attnout[:, mt, ts(nch, NW)], pa[:], COPY, scale=r[:]
            )

    def ow_block(b, attnout, xrt):
        for mt in range(NT):
            for nch in range(NCH):
                ps = ps_conv.tile([P, NW], F32, tag="cps")
                for i in range(2):
                    nc.tensor.matmul(
                        ps[:],
                        wo_sb[:, 2 * i : 2 * i + 2, ts(mt, P)],
                        attnout[:, 2 * i : 2 * i + 2, ts(nch, NW)],
                        start=(i == 0),
                        stop=False,
                        perf_mode=DR,
                    )
                # residual: += 16384 * X  (identity premultiplied by SRES)
                nc.tensor.matmul(
                    ps[:], identr[:], xrt[:, mt, ts(nch, NW)],
                    start=False, stop=True,
                )
                ot = outp.tile([P, NW], BF16, tag="ot")
                nc.scalar.activation(ot[:], ps[:], COPY, scale=1.0 / SRES)
                nc.sync.dma_start(out[b, ts(mt, P), ts(nch, NW)], ot[:])

    pending_ow = None
    for b in range(b_loc):
        x8t = x8_pool.tile([P, NT, PAD_SZ], FP8, tag="x8t")
        nc.gpsimd.dma_start(_r(x8t[:], "p kt s -> p (kt s)"), xp8[b])
        xp8v = _r(x8t[:], "p kt (r c) -> p kt r c", c=PW)
        attnout = att_pool.tile([P, NT, HW], FP8, tag="attnout")
        xrt = None
        pending = None  # deferred attention block for software pipelining

        for mt in range(NT):
            if mt == 1 and pending_ow is not None:
                ow_block(*pending_ow)
                pending_ow = None
            if mt == 1:
                xrt = xr_pool.tile([P, NT, HW], BF16, tag="xrt")
                nc.gpsimd.dma_start(xrt[:], _r(xres[b], "(kt p) n -> p kt n", p=P))
            ydw = {}
            for br in range(3):
                pool = v_pool if br == 2 else qk_pool
                y = pool.tile([P, HW], BF16, tag="v" if br == 2 else "qk")
                if br == 2 and mt in FOLD_V_MT:
                    folded_tile(mt, xp8v, y)
                else:
                    xpad = xpad_pool.tile([P, PAD_SZ], BF16, tag="xpad")
                    xpv = _r(xpad[:], "p (r c) -> p r c", c=PW)
                    # for v the SATT scale rides the conv evacuation (taps
                    # are linear), so y is 256*v in every path
                    conv_scatter(br, mt, x8t, xpad[:])
                    dw_taps(br, mt, xpv, y)
                ydw[br] = y

            qT = qt_pool.tile([P, 32, P], BF16, tag="qT")
            kT = qt_pool.tile([P, 32, P], BF16, tag="kT")
            for qq in range(2):
                nc.sync.dma_start_transpose(
                    qT[:, qq * 16 : (qq + 1) * 16], ydw[0][:, ts(qq, 2048)]
                )
                nc.sync.dma_start_transpose(
                    kT[:, qq * 16 : (qq + 1) * 16], ydw[1][:, ts(qq, 2048)]
                )

            if pending is not None:
                attention(*pending)
            pending = (mt, qT, kT, ydw[2], attnout)

        attention(*pending)
        pending_ow = (b, attnout, xrt)

    ow_block(*pending_ow)

    for p in reversed(pools):
        p.release()


def _fp8(a):
    return np.clip(np.asarray(a, np.float32), -240.0, 240.0).astype(
        ml_dtypes.float8_e4m3
    )


def prep_inputs(style_feat, fw1, fwd_, gw1, gwd, hw1, hwd, ow, temperature):
    """Host-side prep: pad+quantize input, prescale weights, shard over batch."""
    bf16 = ml_dtypes.bfloat16
    sf = np.asarray(style_feat, np.float32)
    temp = np.asarray(temperature, np.float32).reshape(HEADS)

    # padded fp8 input: [B, P, NT*PAD_SZ]
    xpad = np.pad(sf, ((0, 0), (0, 0), (2, 2), (2, 2)), mode="reflect")
    xpad = xpad.reshape(B, NT, P, PAD_SZ).transpose(0, 2, 1, 3).reshape(B, P, NT * PAD_SZ)
    xp8 = _fp8(xpad)

    xres = sf.reshape(B, C, HW).astype(bf16)

    def wT(m, scale):  # [P, NT*C]: [p, kt, o] = m[o, kt*128+p] * scale
        a = (np.asarray(m, np.float32).T * scale).reshape(NT, P, C)
        return a.transpose(1, 0, 2).reshape(P, NT * C)

    wq8 = _fp8(wT(fw1, SW))
    wk8 = _fp8(wT(gw1, SW))
    wv8 = _fp8(wT(hw1, SW))
    wo8 = _fp8(wT(ow, SW))

    # folded v weights: [p, t, kt, o] = hwd[o,t] * hw1[o, kt*128+p] * SFOLD
    wd_v = np.asarray(hwd, np.float32).reshape(C, 9)
    m = np.asarray(hw1, np.float32)
    a = np.einsum("ot,ok->tko", wd_v, m) * SFOLD  # [9, C_in, C_out]
    a = a.reshape(9, NT, P, C).transpose(2, 0, 1, 3).reshape(P, 9 * NT * C)
    wfold8 = _fp8(a)

    # depthwise tap weights -> [128, branch*ctile*9]
    wd_all = np.zeros((P, 3 * NT * 9), dtype=np.float32)
    for bi, wdb in enumerate([fwd_, gwd, hwd]):
        wdb = np.asarray(wdb, np.float32).reshape(C, 9)
        for mt in range(NT):
            wd_all[:, (bi * NT + mt) * 9 : (bi * NT + mt) * 9 + 9] = wdb[
                mt * P : (mt + 1) * P
            ]

    # q evacuation scale: temp per output channel / SW
    tvec = np.repeat(temp, CPH)  # [C]
    sq = (tvec / SW).reshape(NT, P).T.copy()  # [P, NT]

    b_loc = B // N_CORES
    in_maps = []
    for ci in range(N_CORES):
        sl = slice(ci * b_loc, (ci + 1) * b_loc)
        in_maps.append(
            dict(
                xp8=np.ascontiguousarray(xp8[sl]),
                xres=np.ascontiguousarray(xres[sl]),
                wq=wq8, wk=wk8, wv=wv8, wfold=wfold8, wo=wo8,
                wd=wd_all, sq=np.ascontiguousarray(sq),
            )
        )
    return in_maps, b_loc


_CACHED = {}


def _get_module(b_loc):
    if b_loc not in _CACHED:
        _CACHED[b_loc] = build_module(b_loc)
    return _CACHED[b_loc]


def kernel(**inputs):
    in_maps, b_loc = prep_inputs(**inputs)
    nc = _get_module(b_loc)
    res = run_bass_kernel_spmd(nc, in_maps, list(range(N_CORES)))
    outs = [np.asarray(res.results[i]["out"]) for i in range(N_CORES)]
    full = np.concatenate(outs, axis=0).reshape(B, C, H, W)
    return full.astype(np.float32)


if __name__ == "__main__":
    rng = np.random.default_rng(0)
    inputs = dict(
        style_feat=rng.standard_normal((B, C, H, W), dtype=np.float32),
        fw1=(rng.standard_normal((C, C), dtype=np.float32) * 0.02),
        fwd_=(rng.standard_normal((C, 1, 3, 3), dtype=np.float32) * 0.02),
        gw1=(rng.standard_normal((C, C), dtype=np.float32) * 0.02),
        gwd=(rng.standard_normal((C, 1, 3, 3), dtype=np.float32) * 0.02),
        hw1=(rng.standard_normal((C, C), dtype=np.float32) * 0.02),
        hwd=(rng.standard_normal((C, 1, 3, 3), dtype=np.float32) * 0.02),
        ow=(rng.standard_normal((C, C), dtype=np.float32) * 0.02),
        temperature=np.ones((HEADS, 1, 1), dtype=np.float32),
    )
    o = kernel(**inputs)
    print(o.shape, o.dtype)


# revision 19
# speedup vs baseline: 1.5833x; 1.0294x over previous
"""Trainium2 Bass kernel for DilatedMDTA (dense_transformer).

Computation (per batch image X [512, 64, 64]):
  q = DW_f(fw1 @ X) ; k = DW_g(gw1 @ X) ; v = DW_h(hw1 @ X)
  where DW_* is a depthwise 3x3 dilation-2 conv with reflection pad 2.
  energy[h] = q_h @ k_h^T  (contract over the 4096 pixels)
  attn = softmax(energy * temperature, axis=-1)
  out = ow @ (attn @ v) + X

Sharding: data-parallel over batch B=16 across 8 cores (2 images/core).

Per-core mapping (v2):
  - input X is reflection-padded on the host and shipped as fp8 (e4m3);
    all four 1x1 convs run as fp8 DoubleRow matmuls (2 k-tiles/pass).
  - v-branch depthwise conv is FOLDED into the 1x1 conv on the PE for
    mts in FOLD_V_MT: 9 shifted-window accumulation passes with host
    prescaled weights W_t = diag(wd[:,t]) @ W (no elementwise tap work).
  - remaining tiles: conv psum is scattered into a padded SBUF buffer
    (ACT, scale fused), taps split DVE (tensor_scalar mul 4x + add 2x)
    and GPSIMD (scalar_tensor_tensor chain).
  - energy per head-pair as one [128]x[128] PSUM accumulation over 32
    pixel chunks of DMA-transposed qT/kT.
  - softmax: plain exp (logits are O(0.1)), 1/rowsum fused into the
    attn@v PSUM evacuation; attnout emitted as fp8 (x256) so the output
    conv also runs DoubleRow.
  - residual: identity*16384 matmul accumulates X into the ow psum
    (scales: wo x64, attnout x256 -> psum = 16384*(conv+X), evac /16384).
  - output written bf16, upcast on host.
"""

import numpy as np
import ml_dtypes

import concourse.bass as bass
from concourse import bacc
import concourse.mybir as mybir
import concourse.tile as tile
from concourse.bass import ts
from concourse.bass_utils import run_bass_kernel_spmd
from concourse.masks import make_identity

BF16 = mybir.dt.bfloat16
F32 = mybir.dt.float32
FP8 = mybir.dt.float8e4
DR = mybir.MatmulPerfMode.DoubleRow
AX = mybir.AxisListType.X
MUL = mybir.AluOpType.mult
ADD = mybir.AluOpType.add
COPY = mybir.ActivationFunctionType.Copy

N_CORES = 8
B = 16
C = 512
H = W = 64
HW = H * W
HEADS = 8
CPH = C // HEADS  # 64
P = 128
NT = C // P      # 4 channel tiles
NCH = 8          # 512-px chunks per image
NW = HW // NCH   # 512
PW = W + 4       # 68 padded width
PAD_SZ = PW * PW

SW = 64.0        # fp8 scale for unfolded 1x1 weights
SFOLD = 2048.0   # fp8 scale for folded (tap-premultiplied) weights
SATT = 256.0     # attnout scale (fused into v evacuation)
SRES = SW * SATT  # net scale of the ow psum (16384)

FOLD_V_MT = (0, 1, 2)   # v-branch mts whose DW is folded into the PE conv
GPS_TAPS = (6, 7, 8)    # taps done on GPSIMD for elementwise tiles
ACT_TAP0_MT = (0, 2)    # mts whose tap-0 product is offloaded to ACT


def _r(ap, spec, **kw):
    return ap.rearrange(spec, **kw)


def build_module(b_loc: int):
    nc = bacc.Bacc("TRN2", target_bir_lowering=False, debug=False)

    xp8 = nc.dram_tensor("xp8", [b_loc, P, NT * PAD_SZ], FP8, kind="ExternalInput").ap()
    xres = nc.dram_tensor("xres", [b_loc, C, HW], BF16, kind="ExternalInput").ap()
    wq = nc.dram_tensor("wq", [P, NT * C], FP8, kind="ExternalInput").ap()
    wk = nc.dram_tensor("wk", [P, NT * C], FP8, kind="ExternalInput").ap()
    wv = nc.dram_tensor("wv", [P, NT * C], FP8, kind="ExternalInput").ap()
    wfold = nc.dram_tensor("wfold", [P, 9 * NT * C], FP8, kind="ExternalInput").ap()
    wo = nc.dram_tensor("wo", [P, NT * C], FP8, kind="ExternalInput").ap()
    wd = nc.dram_tensor("wd", [P, 3 * NT * 9], F32, kind="ExternalInput").ap()
    sq = nc.dram_tensor("sq", [P, NT], F32, kind="ExternalInput").ap()
    out = nc.dram_tensor("out", [b_loc, C, HW], BF16, kind="ExternalOutput").ap()

    with tile.TileContext(nc) as tc:
        _body(tc, b_loc, xp8, xres, [wq, wk, wv], wfold, wo, wd, sq, out)
    nc.compile()
    return nc


def _body(tc, b_loc, xp8, xres, wqkv, wfold, wo, wd, sq, out):
    nc = tc.nc

    pools = []

    def mkpool(**kw):
        p = tc.alloc_tile_pool(**kw)
        pools.append(p)
        return p

    const = mkpool(name="const", bufs=1)
    x8_pool = mkpool(name="x8", bufs=1)
    xr_pool = mkpool(name="xr", bufs=1)
    xpad_pool = mkpool(name="xpad", bufs=2)
    qk_pool = mkpool(name="qk", bufs=2)
    v_pool = mkpool(name="v", bufs=2)
    qt_pool = mkpool(name="qt", bufs=1)
    att_pool = mkpool(name="att", bufs=2)
    small_pool = mkpool(name="small", bufs=2)
    prod_dve = mkpool(name="prodd", bufs=2)
    prod_act = mkpool(name="proda", bufs=1)
    outp = mkpool(name="outp", bufs=2)
    ps_fold = mkpool(name="ps_fold", bufs=2, space="PSUM")
    ps_conv = mkpool(name="ps_conv", bufs=4, space="PSUM")
    ps_e = mkpool(name="ps_e", bufs=1, space="PSUM")
    ps_t = mkpool(name="ps_t", bufs=1, space="PSUM")

    # weights / consts
    w_sb = []
    for name, wdram in zip("qkv", wqkv):
        t = const.tile([P, NT, C], FP8, tag=f"w{name}")
        nc.gpsimd.dma_start(t[:], _r(wdram, "p (kt o) -> p kt o", kt=NT))
        w_sb.append(t)
    wfold_sb = const.tile([P, 9, NT, C], FP8, tag="wfold")
    nc.gpsimd.dma_start(wfold_sb[:], _r(wfold, "p (t kt o) -> p t kt o", t=9, kt=NT))
    wo_sb = const.tile([P, NT, C], FP8, tag="wo")
    nc.gpsimd.dma_start(wo_sb[:], _r(wo, "p (kt o) -> p kt o", kt=NT))
    wd_sb = const.tile([P, 3 * NT * 9], F32, tag="wd")
    nc.gpsimd.dma_start(wd_sb[:], wd[:])
    sq_sb = const.tile([P, NT], F32, tag="sq")
    nc.gpsimd.dma_start(sq_sb[:], sq[:])
    ident = const.tile([P, P], BF16, tag="ident")
    make_identity(nc, ident[:])
    identr = const.tile([P, P], BF16, tag="identr")
    nc.scalar.mul(identr[:], ident[:], float(SRES))

    def conv_scatter(br, mt, x8t, xpad):
        """1x1 conv of the full PADDED input (pointwise: conv(pad(X)) =
        pad(conv(X))) -> padded buffer with no pad copies at all.
        10 chunks of 7 padded rows (last 5), contiguous in and out."""
        scale = sq_sb[:, mt : mt + 1] if br == 0 else (SATT / SW if br == 2 else 1.0 / SW)
        for ch in range(10):
            r0 = 7 * ch
            ncol = (5 if ch == 9 else 7) * PW
            ps = ps_conv.tile([P, NW], F32, tag="cps")
            for i in range(2):
                nc.tensor.matmul(
                    ps[:, 0:ncol],
                    w_sb[br][:, 2 * i : 2 * i + 2, ts(mt, P)],
                    x8t[:, 2 * i : 2 * i + 2, r0 * PW : r0 * PW + ncol],
                    start=(i == 0),
                    stop=(i == 1),
                    perf_mode=DR,
                )
            nc.scalar.activation(
                xpad[:, r0 * PW : r0 * PW + ncol], ps[:, 0:ncol], COPY, scale=scale
            )

    def dw_taps(br, mt, xpv, y):
        """9-tap depthwise: 2 muls on ACT (incl. the y init), the rest
        muls at 4x + all adds at 2x on DVE. GPSIMD stays idle: its ops
        steal the SBUF port DVE needs for 2x/4x modes."""

        def wsc(t):
            i = (br * NT + mt) * 9 + t
            return wd_sb[:, i : i + 1]

        def srcf(t):
            i, j = t // 3, t % 3
            return xpv[:, 2 * i : 2 * i + H, 2 * j : 2 * j + W]

        yv = _r(y[:], "p (r c) -> p r c", c=W)
        # ACT initializes y with tap 0 and makes the tap-1 product
        nc.scalar.activation(yv, srcf(0), COPY, scale=wsc(0))
        pf0 = prod_act.tile([P, HW], BF16, tag="pf0")
        nc.scalar.activation(
            _r(pf0[:], "p (r c) -> p r c", c=W), srcf(1), COPY, scale=wsc(1)
        )
        for t in range(2, 9):
            pf = prod_dve.tile([P, HW], BF16, tag="pf")
            nc.vector.tensor_scalar_mul(_r(pf[:], "p (r c) -> p r c", c=W), srcf(t), wsc(t))
            nc.vector.tensor_add(y[:], y[:], pf[:])
        nc.vector.tensor_add(y[:], y[:], pf0[:])

    def folded_tile(mt, xp8v, y):
        """v-branch conv+DW fused on the PE: 9 shifted-window DR passes.
        LDWEIGHTS is per-matmul anyway, so accumulate one 512-px chunk
        (1 psum bank) at a time to keep PSUM pressure minimal."""
        for ch in range(NCH):
            ps = ps_fold.tile([P, NW], F32, tag="fps")
            r0 = 8 * ch
            for t in range(9):
                ir, jc = t // 3, t % 3
                for i in range(2):
                    nc.tensor.matmul(
                        _r(ps[:], "p (r c) -> p r c", c=W),
                        wfold_sb[:, t, 2 * i : 2 * i + 2, ts(mt, P)],
                        xp8v[:, 2 * i : 2 * i + 2, r0 + 2 * ir : r0 + 2 * ir + 8, 2 * jc : 2 * jc + W],
                        start=(t == 0 and i == 0),
                        stop=(t == 8 and i == 1),
                        perf_mode=DR,
                    )
            nc.scalar.activation(
                y[:, ts(ch, NW)], ps[:], COPY, scale=SATT / SFOLD
            )

    def attention(mt, qT, kT, v, attnout):
        # energy for head pair (2*mt, 2*mt+1); head-cross blocks unused
        eps = ps_e.tile([P, P], F32, tag="eps")
        for nk in range(32):
            nc.tensor.matmul(
                eps[:], qT[:, nk], kT[:, nk], start=(nk == 0), stop=(nk == 31)
            )
        s = small_pool.tile([P, 1], F32, tag="s")
        lns = small_pool.tile([P, 1], F32, tag="lns")
        r = small_pool.tile([P, 1], F32, tag="r")
        exps = small_pool.tile([P, P], BF16, tag="exps")
        scr = small_pool.tile([P, CPH], BF16, tag="scr")
        # energies here are O(0.1): plain exp is safe, no max subtraction.
        # The whole softmax runs on ACT: row sums via accum_out and
        # 1/sum = exp(-ln(sum)) — keeping DVE's in-order queue free of
        # attention sync points (pure tap stream).
        nc.scalar.activation(
            exps[:], eps[:], mybir.ActivationFunctionType.Exp, bias=0.0, scale=1.0
        )
        for hh in range(2):
            h0 = CPH * hh
            nc.scalar.activation(
                scr[h0 : h0 + CPH, :], exps[h0 : h0 + CPH, h0 : h0 + CPH],
                COPY, accum_out=s[h0 : h0 + CPH],
            )
        nc.scalar.activation(lns[:], s[:], mybir.ActivationFunctionType.Ln)
        nc.scalar.activation(r[:], lns[:], mybir.ActivationFunctionType.Exp, scale=-1.0)

        tps = ps_t.tile([P, P], BF16, tag="tps")
        nc.tensor.transpose(tps[:], exps[:], ident[:])
        attnT = small_pool.tile([P, P], BF16, tag="attnT")
        nc.scalar.copy(attnT[:], tps[:])

        # attn @ v: both heads into one psum bank (concurrent quadrants),
        # single evacuation scaled by 1/rowsum (v carries x256 already)
        for nch in range(NCH):
            pa = ps_conv.tile([P, NW], F32, tag="cps")
            for hh in range(2):
                h0 = CPH * hh
                nc.tensor.matmul(
                    pa[h0 : h0 + CPH],
                    attnT[h0 : h0 + CPH, h0 : h0 + CPH],
                    v[h0 : h0 + CPH, ts(nch, NW)],
                    start=True,
                    stop=True,
                    tile_position=(h0, h0),
                )
            nc.scalar.activation(
                attnout[:, mt, ts(nch, NW)], pa[:], COPY, scale=r[:]
            )

    def ow_block(b, attnout, xrt):
        for mt in range(NT):
            for nch in range(NCH):
                ps = ps_conv.tile([P, NW], F32, tag="cps")
                for i in range(2):
                    nc.tensor.matmul(
                        ps[:],
                        wo_sb[:, 2 * i : 2 * i + 2, ts(mt, P)],
                        attnout[:, 2 * i : 2 * i + 2, ts(nch, NW)],
                        start=(i == 0),
                        stop=False,
                        perf_mode=DR,
                    )
                # residual: += 16384 * X  (identity premultiplied by SRES)
                nc.tensor.matmul(
                    ps[:], identr[:], xrt[:, mt, ts(nch, NW)],
                    start=False, stop=True,
                )
                ot = outp.tile([P, NW], BF16, tag="ot")
                nc.scalar.activation(ot[:], ps[:], COPY, scale=1.0 / SRES)
                nc.sync.dma_start(out[b, ts(mt, P), ts(nch, NW)], ot[:])

    pending_ow = None
    for b in range(b_loc):
        x8t = x8_pool.tile([P, NT, PAD_SZ], FP8, tag="x8t")
        nc.gpsimd.dma_start(_r(x8t[:], "p kt s -> p (kt s)"), xp8[b])
        xp8v = _r(x8t[:], "p kt (r c) -> p kt r c", c=PW)
        attnout = att_pool.tile([P, NT, HW], FP8, tag="attnout")
        xrt = None
        pending = None  # deferred attention block for software pipelining

        for mt in range(NT):
            if mt == 1 and pending_ow is not None:
                ow_block(*pending_ow)
                pending_ow = None
            if mt == 1:
                xrt = xr_pool.tile([P, NT, HW], BF16, tag="xrt")
                nc.gpsimd.dma_start(xrt[:], _r(xres[b], "(kt p) n -> p kt n", p=P))
            ydw = {}
            for br in range(3):
                pool = v_pool if br == 2 else qk_pool
                y = pool.tile([P, HW], BF16, tag="v" if br == 2 else "qk")
                if br == 2 and mt in FOLD_V_MT:
                    folded_tile(mt, xp8v, y)
                else:
                    xpad = xpad_pool.tile([P, PAD_SZ], BF16, tag="xpad")
                    xpv = _r(xpad[:], "p (r c) -> p r c", c=PW)
                    # for v the SATT scale rides the conv evacuation (taps
                    # are linear), so y is 256*v in every path
                    conv_scatter(br, mt, x8t, xpad[:])
                    dw_taps(br, mt, xpv, y)
                ydw[br] = y

            qT = qt_pool.tile([P, 32, P], BF16, tag="qT")
            kT = qt_pool.tile([P, 32, P], BF16, tag="kT")
            for qq in range(2):
                nc.sync.dma_start_transpose(
                    qT[:, qq * 16 : (qq + 1) * 16], ydw[0][:, ts(qq, 2048)]
                )
                nc.sync.dma_start_transpose(
                    kT[:, qq * 16 : (qq + 1) * 16], ydw[1][:, ts(qq, 2048)]
                )

            if pending is not None:
                attention(*pending)
            pending = (mt, qT, kT, ydw[2], attnout)

        attention(*pending)
        pending_ow = (b, attnout, xrt)

    ow_block(*pending_ow)

    for p in reversed(pools):
        p.release()


def _fp8(a):
    return np.clip(np.asarray(a, np.float32), -240.0, 240.0).astype(
        ml_dtypes.float8_e4m3
    )


def prep_inputs(style_feat, fw1, fwd_, gw1, gwd, hw1, hwd, ow, temperature):
    """Host-side prep: pad+quantize input, prescale weights, shard over batch."""
    bf16 = ml_dtypes.bfloat16
    sf = np.asarray(style_feat, np.float32)
    temp = np.asarray(temperature, np.float32).reshape(HEADS)

    # padded fp8 input: [B, P, NT*PAD_SZ]
    xpad = np.pad(sf, ((0, 0), (0, 0), (2, 2), (2, 2)), mode="reflect")
    xpad = xpad.reshape(B, NT, P, PAD_SZ).transpose(0, 2, 1, 3).reshape(B, P, NT * PAD_SZ)
    xp8 = _fp8(xpad)

    xres = sf.reshape(B, C, HW).astype(bf16)

    def wT(m, scale):  # [P, NT*C]: [p, kt, o] = m[o, kt*128+p] * scale
        a = (np.asarray(m, np.float32).T * scale).reshape(NT, P, C)
        return a.transpose(1, 0, 2).reshape(P, NT * C)

    wq8 = _fp8(wT(fw1, SW))
    wk8 = _fp8(wT(gw1, SW))
    wv8 = _fp8(wT(hw1, SW))
    wo8 = _fp8(wT(ow, SW))

    # folded v weights: [p, t, kt, o] = hwd[o,t] * hw1[o, kt*128+p] * SFOLD
    wd_v = np.asarray(hwd, np.float32).reshape(C, 9)
    m = np.asarray(hw1, np.float32)
    a = np.einsum("ot,ok->tko", wd_v, m) * SFOLD  # [9, C_in, C_out]
    a = a.reshape(9, NT, P, C).transpose(2, 0, 1, 3).reshape(P, 9 * NT * C)
    wfold8 = _fp8(a)

    # depthwise tap weights -> [128, branch*ctile*9]
    wd_all = np.zeros((P, 3 * NT * 9), dtype=np.float32)
    for bi, wdb in enumerate([fwd_, gwd, hwd]):
        wdb = np.asarray(wdb, np.float32).reshape(C, 9)
        for mt in range(NT):
            wd_all[:, (bi * NT + mt) * 9 : (bi * NT + mt) * 9 + 9] = wdb[
                mt * P : (mt + 1) * P
            ]

    # q evacuation scale: temp per output channel / SW
    tvec = np.repeat(temp, CPH)  # [C]
    sq = (tvec / SW).reshape(NT, P).T.copy()  # [P, NT]

    b_loc = B // N_CORES
    in_maps = []
    for ci in range(N_CORES):
        sl = slice(ci * b_loc, (ci + 1) * b_loc)
        in_maps.append(
            dict(
                xp8=np.ascontiguousarray(xp8[sl]),
                xres=np.ascontiguousarray(xres[sl]),
                wq=wq8, wk=wk8, wv=wv8, wfold=wfold8, wo=wo8,
                wd=wd_all, sq=np.ascontiguousarray(sq),
            )
        )
    return in_maps, b_loc


_CACHED = {}


def _get_module(b_loc):
    if b_loc not in _CACHED:
        _CACHED[b_loc] = build_module(b_loc)
    return _CACHED[b_loc]


def kernel(**inputs):
    in_maps, b_loc = prep_inputs(**inputs)
    nc = _get_module(b_loc)
    res = run_bass_kernel_spmd(nc, in_maps, list(range(N_CORES)))
    outs = [np.asarray(res.results[i]["out"]) for i in range(N_CORES)]
    full = np.concatenate(outs, axis=0).reshape(B, C, H, W)
    return full.astype(np.float32)


if __name__ == "__main__":
    rng = np.random.default_rng(0)
    inputs = dict(
        style_feat=rng.standard_normal((B, C, H, W), dtype=np.float32),
        fw1=(rng.standard_normal((C, C), dtype=np.float32) * 0.02),
        fwd_=(rng.standard_normal((C, 1, 3, 3), dtype=np.float32) * 0.02),
        gw1=(rng.standard_normal((C, C), dtype=np.float32) * 0.02),
        gwd=(rng.standard_normal((C, 1, 3, 3), dtype=np.float32) * 0.02),
        hw1=(rng.standard_normal((C, C), dtype=np.float32) * 0.02),
        hwd=(rng.standard_normal((C, 1, 3, 3), dtype=np.float32) * 0.02),
        ow=(rng.standard_normal((C, C), dtype=np.float32) * 0.02),
        temperature=np.ones((HEADS, 1, 1), dtype=np.float32),
    )
    o = kernel(**inputs)
    print(o.shape, o.dtype)
